# revision 13
# baseline (speedup 1.0000x reference)
"""Trainium2 Bass kernel for nn_Evolution_26697516712465 (deep-snake GNN).

Self-contained: takes FULL inputs, shards batch across 8 NeuronCores internally
(one image per core; each core runs the snake for the polys of its own image),
returns FULL output [128, 128, 2] fp32.

fp8e4 (e4m3) DoubleRow matmuls throughout (2 contraction rows/cycle), weights
pre-scaled by 64 into fp8 normal range, activations carried at power-of-2
scales; bilinear grid-sample folded into PE "diagonal" matmuls on gathered
corner row-pairs; eval-mode bn folded into weights/biases host-side.
"""
import numpy as np
import ml_dtypes
from contextlib import ExitStack

import concourse.bass as bass
import concourse.bacc as bacc
import concourse.mybir as mybir
import concourse.tile as tile
from concourse.library_config import mlp as mlp_lib
from concourse.bass_utils import run_bass_kernel_spmd

N_CORES = 8
B, C_IN, H, W = 8, 66, 128, 128
NP, V = 128, 128
RO = 4.0
DIL = (1, 1, 1, 2, 2, 4, 4)
NRES = 7
HW = H * W          # 16384
PADW = W + 2        # 130
PIMG = PADW * PADW  # 16900
PADV = 160          # 16 + 128 + 16 circular pad

f32 = mybir.dt.float32
f32r = mybir.dt.float32r
fp8 = mybir.dt.float8e4
i16 = mybir.dt.int16
AF = mybir.ActivationFunctionType
ALU = mybir.AluOpType
DR = mybir.MatmulPerfMode.DoubleRow

F8 = ml_dtypes.float8_e4m3

# activation/weight scales (powers of 2)
A_W = 64.0          # weight scale
S_R1 = 8.0          # conv1 relu out
S_FEAT = 32.0       # ipad feat rows (folded into diag weights)
S_Z = 32.0          # snake states
S_GB = 128.0        # fusion global feature
S_H1 = 128.0
S_H2 = 512.0

# conv1 stack row maps: blkA = 66ch kh0 + 33ch kh1; blkB = 33ch kh1 + 66ch kh2
ROWMAP_A = [(r, 0) if r < 66 else (r - 66, 1) for r in range(99)]
ROWMAP_B = [(r + 33, 1) if r < 33 else (r - 33, 2) for r in range(99)]


def _f8(x):
    return np.clip(np.asarray(x, np.float32), -240.0, 240.0).astype(F8)


def pack16(idx_flat, cols):
    tab = np.zeros((16, cols), np.int16)
    n = len(idx_flat)
    tab[np.arange(n) % 16, np.arange(n) // 16] = idx_flat.astype(np.int16)
    return np.tile(tab, (8, 1))


def build_nc(P, with_b2, zb):
    """Build the SPMD Bass program. P = max polys per image; zb = all relu
    biases are zero (allows relu on DVE/Pool engines)."""
    nc = bacc.Bacc("TRN2", target_bir_lowering=False, debug=False)
    PADQ = -(-P // 4) * 4
    NQB = PADQ // 4
    NV = PADQ * 128

    # ---------------- inputs ----------------
    # blobA: w1p | w2t | headw | pw3 (fp8, conv+early weights)
    CA_W1P, CA_W2T, CA_HW, CA_PW3 = 0, 1536, 1664, 2944
    CA = 2946
    # blobB: dgt | resw | fusw | pw1 | pw2 (fp8, late weights)
    CB_DGT = 0
    CB_RES = PADQ * 512
    CB_FUS = CB_RES + 8960
    CB_PW1 = CB_FUS + 2048
    CB_PW2 = CB_PW1 + 2560
    CB = CB_PW2 + 128
    # consts (f32): pb0 | lsb | fusc | pb1 | pb2 | base
    CC_PB0, CC_LSB, CC_FUSC, CC_PB1, CC_PB2, CC_BASE = 0, 2, 18, 20, 22, 24
    CC = 24 + PADQ * 2
    d_stk = nc.declare_dram_parameter("stk", [99, 2, PIMG], fp8, isOutput=False)
    d_blobA = nc.declare_dram_parameter("blobA", [128, CA], fp8, isOutput=False)
    d_blobB = nc.declare_dram_parameter("blobB", [128, CB], fp8, isOutput=False)
    d_consts = nc.declare_dram_parameter("consts", [128, CC], f32, isOutput=False)
    d_gix = nc.declare_dram_parameter("gix", [128, 2 * (NV // 16)], i16, isOutput=False)
    d_cpv = nc.declare_dram_parameter("cpv", [2, PADQ, 160], fp8, isOutput=False)
    if with_b2:
        d_b2r = nc.declare_dram_parameter("b2r", [1, 64], fp8, isOutput=False)
        d_svr = nc.declare_dram_parameter("svr", [1, PADQ, 128], fp8, isOutput=False)
    d_out = nc.declare_dram_parameter("out", [128, PADQ, 2], f32, isOutput=True)

    feat_dram = nc.dram_tensor("feat_dram", [HW, 64], f32)

    with tile.TileContext(nc, num_cores=N_CORES) as tc, ExitStack() as top:
        wpool = top.enter_context(tc.tile_pool(name="weights", bufs=1))
        # SP queue: blobA, consts, gix (small, early)
        blobA = wpool.tile([128, CA], fp8)
        nc.sync.dma_start(out=blobA, in_=d_blobA[:, :])
        consts = wpool.tile([128, CC], f32)
        nc.sync.dma_start(out=consts, in_=d_consts[:, :])
        gix_t = wpool.tile([128, 2 * (NV // 16)], i16)
        nc.sync.dma_start(out=gix_t, in_=d_gix[:, :])
        if with_b2:
            b2r_t = wpool.tile([1, 64], fp8)
            nc.sync.dma_start(out=b2r_t, in_=d_b2r[:, :])
            svr_t = wpool.tile([1, PADQ, 128], fp8)
            nc.sync.dma_start(out=svr_t, in_=d_svr[:, :, :])

        nc.gpsimd.load_library(mlp_lib)

        # Pool DMA queue: stk bands, then blobB chunks (dgt first)
        stk_t = wpool.tile([99, 2, PIMG], fp8)
        CHK = 16 * PADW
        for bb in range(8):
            nc.sync.dma_start(out=stk_t[:, :, bb * CHK:(bb + 1) * CHK],
                              in_=d_stk[:, :, bb * CHK:(bb + 1) * CHK])
        blobB = wpool.tile([128, CB], fp8)
        nc.gpsimd.dma_start(out=blobB[:, CB_DGT:CB_RES],
                            in_=d_blobB[:, CB_DGT:CB_RES])
        for i in range(NRES):
            c0 = CB_RES + i * 1280
            nc.gpsimd.dma_start(out=blobB[:, c0:c0 + 1280],
                                in_=d_blobB[:, c0:c0 + 1280])
        nc.gpsimd.dma_start(out=blobB[:, CB_FUS:CB],
                            in_=d_blobB[:, CB_FUS:CB])

        aps = blobA.ap[0][0]
        bps = blobB.ap[0][0]
        cps = consts.ap[0][0]

        def ap_of(blob, pstride, parts, col0, dims):
            return bass.AP(tensor=blob.tensor, offset=blob.offset + col0,
                           ap=[[pstride, parts]] + dims)

        def w1p_ap(p, m):
            return ap_of(blobA, aps, 99, CA_W1P + p * 512 + m * 128,
                         [[256, 2], [1, 128]])

        def w2t_ap():
            return ap_of(blobA, aps, 128, CA_W2T, [[64, 2], [1, 64]])

        def headw_ap(p):
            return ap_of(blobA, aps, 66, CA_HW + p * 256, [[128, 2], [1, 128]])

        def pw3_ap():
            return ap_of(blobA, aps, 64, CA_PW3, [[1, 2]])

        def dgt_ap(q, gi):
            return ap_of(blobB, bps, 128, CB_DGT + q * 512 + gi * 256,
                         [[128, 2], [1, 128]])

        def resw_ap(i, p, m):
            return ap_of(blobB, bps, 128, CB_RES + i * 1280 + p * 256,
                         [[128, 2], [1, 128]])

        def fusw_ap(k, m):
            return ap_of(blobB, bps, 128, CB_FUS + k * 512 + m * 128,
                         [[256, 2], [1, 128]])

        def pw1_ap(p, m):
            return ap_of(blobB, bps, 128, CB_PW1 + p * 512 + m * 128,
                         [[256, 2], [1, 128]])

        def pw2_ap():
            return ap_of(blobB, bps, 128, CB_PW2, [[64, 2], [1, 64]])

        def const_col(c0, parts=128, n=1):
            return ap_of(consts, cps, parts, c0, [[1, n]])

        base_ap = ap_of(consts, cps, 128, CC_BASE, [[2, PADQ], [1, 2]])

        # --- engine helpers ---
        rr_state = [0]

        def rr_relu(out_ap, in_ap, scale, bias_ap, force=None):
            """out = relu(scale*x + bias). scale may be const or AP.
            If zb (bias==0), can run on act or dve; else activation only.
            (GPSIMD cannot access PSUM, so pool never does these.)"""
            if not zb or force == 'act':
                nc.scalar.activation(out_ap, in_ap, AF.Relu,
                                     bias=(bias_ap if (bias_ap is not None and
                                                       not zb) else 0.0),
                                     scale=scale)
                return
            e = force if force is not None else ('act', 'dve')[rr_state[0] % 2]
            if force is None:
                rr_state[0] += 1
            if e == 'act':
                nc.scalar.activation(out_ap, in_ap, AF.Relu, bias=0.0,
                                     scale=scale)
            else:
                nc.vector.tensor_scalar(out_ap, in_ap, scale, 0.0,
                                        op0=ALU.mult, op1=ALU.max)

        def rr_copy(out_ap, in_ap, scale=None, force=None, pool_ok=False):
            engs = ('act', 'dve', 'pool') if pool_ok else ('act', 'dve')
            e = force if force is not None else engs[rr_state[0] % len(engs)]
            if force is None:
                rr_state[0] += 1
            if scale is None:
                if e == 'act':
                    nc.scalar.activation(out_ap, in_ap, AF.Copy, bias=0.0)
                elif e == 'dve':
                    nc.vector.tensor_copy(out_ap, in_ap)
                else:
                    nc.gpsimd.tensor_copy(out_ap, in_ap)
            else:
                if e == 'act':
                    nc.scalar.activation(out_ap, in_ap, AF.Copy, bias=0.0,
                                         scale=scale)
                elif e == 'dve':
                    nc.vector.tensor_scalar(out_ap, in_ap, scale, None,
                                            op0=ALU.mult)
                else:
                    nc.gpsimd.tensor_scalar(out_ap, in_ap, scale, None,
                                            op0=ALU.mult)

        # ------------ conv1 (3x3, 66->256) + conv2 (1x1, 256->64) ------------
        with tc.tile_pool(name="conv", bufs=1) as cpool, \
             tc.tile_pool(name="psumA", bufs=2, space="PSUM") as ppA, \
             tc.tile_pool(name="psumB", bufs=2, space="PSUM") as ppB, \
             tc.tile_pool(name="stage", bufs=3) as spool:
            r1 = cpool.tile([128, 2, HW], fp8)
            ps2 = {}

            def emit_conv2(g):
                h = g % 2
                if h == 0:
                    ps2[0] = ppB.tile([128, 8, 64], f32, tag="c2", name="c2")
                for cc in range(4):
                    pos0 = g * 512 + cc * 128
                    lhsT = bass.AP(tensor=r1.tensor, offset=r1.offset + pos0,
                                   ap=[r1.ap[0], [HW, 2], [1, 128]])
                    nc.tensor.matmul(ps2[0][:, h * 4 + cc, :], lhsT, w2t_ap(),
                                     start=True, stop=True, perf_mode=DR)
                if h == 1:
                    stg = spool.tile([128, 8, 64], f32, tag="stage", name="stg")
                    rr_copy(stg, ps2[0], 1.0 / (A_W * S_R1))
                    dst = bass.AP(tensor=feat_dram,
                                  offset=((g - 1) * 512) * 64,
                                  ap=[[64, 128], [8192, 8], [1, 64]])
                    nc.sync.dma_start(out=dst, in_=stg)

            for g in range(32):              # y-groups of 4 rows
                for m in range(2):
                    pg = ppA.tile([128, 4, 128], f32, tag=f"c1_{m}",
                                  name=f"c1_{m}")
                    for yy in range(4):
                        y = 4 * g + yy
                        ktaps = ((y * PADW, 1), (y * PADW + 2, PIMG - 2),
                                 (PIMG + y * PADW + 1, 1))
                        for p, (off, stride) in enumerate(ktaps):
                            rhs = bass.AP(tensor=stk_t.tensor,
                                          offset=stk_t.offset + off,
                                          ap=[stk_t.ap[0], [stride, 2],
                                              [1, 128]])
                            nc.tensor.matmul(pg[:, yy, :], w1p_ap(p, m),
                                             rhs, start=(p == 0), stop=(p == 2),
                                             perf_mode=DR)
                    rr_relu(r1[:, m, g * 512:(g + 1) * 512],
                            pg.rearrange("p a b -> p (a b)"), S_R1 / A_W,
                            const_col(CC_PB0 + m))
                if g >= 2:
                    emit_conv2(g - 2)        # skewed to avoid PE queue stall
            emit_conv2(30)
            emit_conv2(31)

        # ------------ gather + combine-transpose into ipad ------------
        with tc.tile_pool(name="snake", bufs=1) as sn:
            ipad = sn.tile([128, PADQ, PADV], fp8, tag="ipad", name="ipad")
            zall = sn.tile([128, 8, PADQ, PADV], fp8, tag="zall", name="zall")
            rsc = sn.tile([128, PADQ, 128], fp8, tag="rsc", name="rsc")

            with tc.tile_pool(name="gat", bufs=1) as gp, \
                 tc.tile_pool(name="psumG", bufs=6, space="PSUM") as ppG:
                gta = gp.tile([128, PADQ, 128], f32, tag="gta", name="gta")
                gtb = gp.tile([128, PADQ, 128], f32, tag="gtb", name="gtb")
                gsrc = bass.AP(tensor=feat_dram, offset=0,
                               ap=[[64, HW - 1], [1, 128]])
                nc.gpsimd.dma_gather(gta, gsrc, gix_t[:, 0:NV // 16], NV, NV,
                                     128, elem_step=64, single_packet=False)
                nc.gpsimd.dma_gather(gtb, gsrc, gix_t[:, NV // 16:], NV, NV,
                                     128, elem_step=64, single_packet=False)
                nc.sync.dma_start(out=ipad[64:66, :, :], in_=d_cpv[:, :, :])

                # convert gathered corners f32 -> fp8 (x S_FEAT)
                g8a = gp.tile([128, PADQ, 128], fp8, tag="g8a", name="g8a")
                g8b = gp.tile([128, PADQ, 128], fp8, tag="g8b", name="g8b")
                for src, dst in ((gta, g8a), (gtb, g8b)):
                    for qb in range(NQB):
                        qsl = slice(4 * qb, 4 * qb + 4)
                        rr_copy(dst[:, qsl, :], src[:, qsl, :],
                                scale=S_FEAT, pool_ok=True)

                for qb in range(NQB):
                    pg = ppG.tile([64, 4, 128], f32, tag="dg", name="dg")
                    for qq in range(4):
                        q = 4 * qb + qq
                        last = 2 if with_b2 else 1
                        for gi, gt in enumerate((g8a, g8b)):
                            lhsT = bass.AP(
                                tensor=gt.tensor,
                                offset=gt.offset + q * 128,
                                ap=[gt.ap[0], [64, 2], [1, 64]])
                            nc.tensor.matmul(pg[:, qq, :], lhsT,
                                             dgt_ap(q, gi),
                                             start=(gi == 0), stop=(gi == last),
                                             perf_mode=DR)
                        if with_b2:
                            nc.tensor.matmul(pg[:, qq, :], b2r_t[:, :],
                                             svr_t[:, q, :], start=False,
                                             stop=True)
                    rr_copy(ipad[0:64, 4 * qb:4 * qb + 4, 16:144], pg)
                nc.vector.tensor_copy(ipad[0:66, :, 0:16],
                                      ipad[0:66, :, 128:144])
                nc.gpsimd.tensor_copy(ipad[0:66, :, 144:160],
                                      ipad[0:66, :, 16:32])

            # ------------ snake ------------
            with tc.tile_pool(name="psumS", bufs=4, space="PSUM") as ppS:

                def conv_layer(zo, rhs_base_fn, rhs_tensor, rhs_ap0, lhsT_fn,
                               dil, src_zi):
                    """One circular conv layer; per-qb relu (+residual add)."""
                    for qb in range(NQB):
                        ps = ppS.tile([128, 4, 128], f32, tag="psS", name="psS")
                        for qq in range(4):
                            q = 4 * qb + qq
                            for p in range(5):
                                if p < 4:
                                    off = rhs_base_fn(q) + 16 + (2 * p - 4) * dil
                                    stride = dil
                                else:
                                    off = rhs_base_fn(q) + 16 + 4 * dil
                                    stride = 0
                                rhs = bass.AP(tensor=rhs_tensor, offset=off,
                                              ap=[rhs_ap0, [stride, 2],
                                                  [1, 128]])
                                nc.tensor.matmul(ps[:, qq, :], lhsT_fn(p), rhs,
                                                 start=(p == 0), stop=(p == 4),
                                                 perf_mode=DR)
                        qsl = slice(4 * qb, 4 * qb + 4)
                        scale_ap = const_col(CC_LSB + 2 * zo)
                        bias_ap = const_col(CC_LSB + 2 * zo + 1)
                        if src_zi is None:
                            # head: write z0 directly
                            rr_relu(zall[:, 0, qsl, 16:144], ps, scale_ap,
                                    bias_ap, force=('act' if qb != 4 else 'dve'))
                        else:
                            rr_relu(rsc[:, qsl, :], ps, scale_ap, bias_ap,
                                    force=('act' if qb != 4 else 'dve'))
                            addeng = nc.gpsimd if qb in (1, 2) else nc.vector
                            addeng.tensor_tensor(
                                zall[:, zo, qsl, 16:144],
                                zall[:, src_zi, qsl, 16:144],
                                rsc[:, qsl, :], ALU.add)
                        if qb == 0:
                            # urgent wraps so next layer's qb0 can start
                            nc.vector.tensor_copy(zall[:, zo, 0:4, 0:16],
                                                  zall[:, zo, 0:4, 128:144])
                            nc.vector.tensor_copy(zall[:, zo, 0:4, 144:160],
                                                  zall[:, zo, 0:4, 16:32])
                    nc.vector.tensor_copy(zall[:, zo, 4:PADQ, 0:16],
                                          zall[:, zo, 4:PADQ, 128:144])
                    nc.gpsimd.tensor_copy(zall[:, zo, 4:PADQ, 144:160],
                                          zall[:, zo, 4:PADQ, 16:32])

                ip66 = ipad[0:66, :, :]
                conv_layer(0, lambda q: ip66.offset + q * PADV, ip66.tensor,
                           ip66.ap[0], headw_ap, 1, None)
                for i in range(NRES):
                    zi_off = zall.offset + i * PADQ * PADV
                    conv_layer(i + 1,
                               lambda q, zi_off=zi_off: zi_off + q * PADV,
                               zall.tensor, zall.ap[0],
                               lambda p, i=i: resw_ap(i, p, 0),
                               DIL[i], i)

                # fusion 1x1 (1024->256) + per-poly max over V
                gmax = sn.tile([128, 2, PADQ], f32, tag="gmax", name="gmax")
                gb = sn.tile([128, 2, PADQ], fp8, tag="gb", name="gb")
                for m in range(2):
                    for qb in range(NQB):
                        ps = ppS.tile([128, 4, 128], f32, tag="psS", name="psS")
                        for qq in range(4):
                            q = 4 * qb + qq
                            for k in range(4):
                                off = zall.offset + (2 * k * PADQ + q) * PADV + 16
                                rhs = bass.AP(tensor=zall.tensor, offset=off,
                                              ap=[zall.ap[0],
                                                  [PADQ * PADV, 2], [1, 128]])
                                nc.tensor.matmul(ps[:, qq, :],
                                                 fusw_ap(k, m), rhs,
                                                 start=(k == 0), stop=(k == 3),
                                                 perf_mode=DR)
                        nc.vector.tensor_reduce(gmax[:, m, 4 * qb:4 * qb + 4],
                                                ps, axis=mybir.AxisListType.X,
                                                op=ALU.max)
                    nc.vector.tensor_scalar(gb[:, m, :], gmax[:, m, :],
                                            S_GB / (A_W * S_Z),
                                            const_col(CC_FUSC + m),
                                            op0=ALU.mult, op1=ALU.add)

                # pred1: 1280 -> 256 relu
                h1 = sn.tile([128, 2, PADQ, 128], fp8, tag="h1", name="h1")
                for m in range(2):
                    for qb in range(NQB):
                        ps = ppS.tile([128, 4, 128], f32, tag="psS", name="psS")
                        for qq in range(4):
                            q = 4 * qb + qq
                            for k in range(4):
                                off = zall.offset + (2 * k * PADQ + q) * PADV + 16
                                rhs = bass.AP(tensor=zall.tensor, offset=off,
                                              ap=[zall.ap[0],
                                                  [PADQ * PADV, 2], [1, 128]])
                                nc.tensor.matmul(ps[:, qq, :],
                                                 pw1_ap(k + 1, m), rhs,
                                                 start=(k == 0), stop=False,
                                                 perf_mode=DR)
                            rhs0 = bass.AP(tensor=gb.tensor,
                                           offset=gb.offset + q,
                                           ap=[gb.ap[0], [PADQ, 2], [0, 128]])
                            nc.tensor.matmul(ps[:, qq, :], pw1_ap(0, m),
                                             rhs0, start=False, stop=True,
                                             perf_mode=DR)
                        rr_relu(h1[:, m, 4 * qb:4 * qb + 4, :], ps,
                                S_H1 / (A_W * S_Z), const_col(CC_PB1 + m))

            # pred2 + pred3
            with tc.tile_pool(name="psumT", bufs=3, space="PSUM") as ppT:
                h2 = sn.tile([64, PADQ, 128], fp8, tag="h2", name="h2")
                for qb in range(NQB):
                    ps = ppT.tile([64, 4, 128], f32, tag="psT", name="psT")
                    for qq in range(4):
                        q = 4 * qb + qq
                        rhs = bass.AP(tensor=h1.tensor,
                                      offset=h1.offset + q * 128,
                                      ap=[h1.ap[0], [PADQ * 128, 2], [1, 128]])
                        nc.tensor.matmul(ps[:, qq, :], pw2_ap(), rhs,
                                         start=True, stop=True, perf_mode=DR)
                    rr_relu(h2[:, 4 * qb:4 * qb + 4, :], ps,
                            S_H2 / (A_W * S_H1), const_col(CC_PB2, parts=64))

                ps3 = ppT.tile([128, PADQ, 2], f32, tag="psT3", name="psT3",
                               bufs=1)
                for q in range(PADQ):
                    nc.tensor.matmul(ps3[:, q, :], h2[:, q, :], pw3_ap(),
                                     start=True, stop=True)
                o_f = sn.tile([128, PADQ, 2], f32, tag="o_f", name="o_f")
                nc.vector.tensor_scalar(o_f, ps3, 1.0 / (A_W * S_H2), None,
                                        op0=ALU.mult)
                o_t = sn.tile([128, PADQ, 2], f32, tag="o_t", name="o_t")
                nc.vector.tensor_tensor(o_t, o_f, base_ap, ALU.add)
                nc.sync.dma_start(out=d_out[:, :, :], in_=o_t)

    nc.compile()
    return nc


_NC_CACHE = {}


def _get_nc_key(P, with_b2, zb):
    key = (P, with_b2, zb)
    if key not in _NC_CACHE:
        _NC_CACHE[key] = build_nc(P, with_b2, zb)
    return _NC_CACHE[key]


def _get_nc(P):
    """test.py compatibility: default flags for the standard input set."""
    return _get_nc_key(P, False, True)


def _host_prep(inputs, P, counts, order, offs):
    """Build per-core in_maps. Returns (in_maps, with_b2, zb)."""
    PADQ = -(-P // 4) * 4
    NV = PADQ * 128
    cnn = np.asarray(inputs["cnn_feature"], np.float32)
    ipoly = np.asarray(inputs["i_it_poly"], np.float32)
    cpoly = np.asarray(inputs["c_it_poly"], np.float32)
    w1 = np.asarray(inputs["proj_w1"], np.float32)
    pb0 = np.asarray(inputs["proj_b1"], np.float32)
    w2 = np.asarray(inputs["proj_w2"], np.float32)[:, :, 0, 0]  # [64, 256]
    b2 = np.asarray(inputs["proj_b2"], np.float32)
    head_w = np.asarray(inputs["head_w"], np.float32)   # [128, 66, 9]
    head_b = np.asarray(inputs["head_b"], np.float32)
    head_g = np.asarray(inputs["head_g"], np.float32)
    head_bt = np.asarray(inputs["head_bt"], np.float32)
    res_w = np.asarray(inputs["res_w"], np.float32)     # [7, 128, 128, 9]
    res_b = np.asarray(inputs["res_b"], np.float32)
    res_g = np.asarray(inputs["res_g"], np.float32)
    res_bt = np.asarray(inputs["res_bt"], np.float32)
    fus_w = np.asarray(inputs["fus_w"], np.float32)     # [256, 1024]
    fus_b = np.asarray(inputs["fus_b"], np.float32)
    pw1 = np.asarray(inputs["pw1"], np.float32)         # [256, 1280]
    pb1 = np.asarray(inputs["pb1"], np.float32)
    pw2 = np.asarray(inputs["pw2"], np.float32)         # [64, 256]
    pb2 = np.asarray(inputs["pb2"], np.float32)
    pw3 = np.asarray(inputs["pw3"], np.float32)         # [2, 64]
    pb3 = np.asarray(inputs["pb3"], np.float32)

    assert (head_g > 0).all() and (res_g > 0).all(), "bn fold requires g>0"
    with_b2 = bool(np.any(b2 != 0))

    # w1p [99, 3pairs, 2kt, 2m, 128]
    w1p = np.zeros((99, 3, 2, 2, 128), np.float32)
    pair_src = [((0, 0), (0, 1)), ((0, 2), (1, 0)), ((1, 1), (1, 2))]
    for p, pr in enumerate(pair_src):
        for kt, (blk, kw) in enumerate(pr):
            rm = ROWMAP_A if blk == 0 else ROWMAP_B
            for r in range(99):
                ch, kh = rm[r]
                for m in range(2):
                    w1p[r, p, kt, m, :] = A_W * w1[m * 128:(m + 1) * 128,
                                                   ch, kh, kw]
    pb0s = (S_R1 * pb0).reshape(2, 128).T.copy()

    w2t = np.zeros((128, 2, 64), np.float32)
    for kt in range(2):
        w2t[:, kt, :] = A_W * w2[:, kt * 128:(kt + 1) * 128].T

    # ---- grid-sample host math ----
    ix = ipoly[..., 0] - np.float32(0.5)
    iy = ipoly[..., 1] - np.float32(0.5)
    x0 = np.floor(ix); y0 = np.floor(iy)
    wx = (ix - x0).astype(np.float32); wy = (iy - y0).astype(np.float32)
    x0i = x0.astype(np.int64); y0i = y0.astype(np.int64)

    swap_x = x0i < 0
    vx0 = (x0i >= 0) & (x0i < W)
    vx1 = (x0i + 1 >= 0) & (x0i + 1 < W)

    def slot_weights(yi):
        vy = (yi >= 0) & (yi < H)
        w_s0 = (1 - wx) * vx0 * vy
        w_s1 = wx * vx1 * vy
        w_s0 = np.where(swap_x, wx * vx1 * vy, w_s0)
        w_s1 = np.where(swap_x, 0.0, w_s1)
        return w_s0.astype(np.float32), w_s1.astype(np.float32)

    x0c = np.clip(x0i, 0, W - 2)
    y0c = np.clip(y0i, 0, H - 1)
    y1c = np.clip(y0i + 1, 0, H - 1)
    idxA = (y0c * W + x0c).astype(np.int64)          # [NP, V]
    idxB = (y1c * W + x0c).astype(np.int64)
    wA0, wA1 = slot_weights(y0i)
    wB0, wB1 = slot_weights(y0i + 1)
    wA0 *= (1 - wy); wA1 *= (1 - wy)
    wB0 *= wy; wB1 *= wy
    s_v = wA0 + wA1 + wB0 + wB1

    # ---- snake weights (bn + scale folds) ----
    headw = np.zeros((66, 5, 2, 128), np.float32)
    hw9 = head_w.transpose(1, 2, 0)                  # [66, 9, 128]
    for t in range(9):
        p, kt = t // 2, t % 2
        headw[0:64, p, kt, :] = A_W * hw9[0:64, t, :]
        headw[64:66, p, kt, :] = A_W * S_FEAT * hw9[64:66, t, :]
    lsb = np.zeros((128, 8, 2), np.float32)
    lsb[:, 0, 0] = head_g / A_W
    lsb[:, 0, 1] = S_Z * head_g * head_b
    C = np.zeros((8, 128), np.float32)               # C_i = sum_{j<=i} bt_j
    C[0] = head_bt
    for i in range(NRES):
        C[i + 1] = C[i] + res_bt[i]

    resw = np.zeros((128, 7, 5, 2, 128), np.float32)
    for i in range(NRES):
        rw = res_w[i].transpose(1, 2, 0)             # [128 in, 9, 128 out]
        for t in range(9):
            p, kt = t // 2, t % 2
            resw[:, i, p, kt, :] = A_W * rw[:, t, :]
        bprime = res_b[i] + res_w[i].sum(axis=2) @ C[i]
        lsb[:, i + 1, 0] = res_g[i] / A_W
        lsb[:, i + 1, 1] = S_Z * res_g[i] * bprime

    fw8 = fus_w.reshape(256, 8, 128)
    fusw = np.zeros((128, 4, 2, 2, 128), np.float32)
    for k in range(4):
        for kt in range(2):
            for m in range(2):
                fusw[:, k, kt, m, :] = A_W * fw8[m * 128:(m + 1) * 128,
                                                 2 * k + kt].T
    fusconst = fus_b + np.einsum('ojc,jc->o', fw8, C)
    fusc = (S_GB * fusconst).reshape(2, 128).T.copy()

    pw1r = pw1.reshape(256, 10, 128)
    pw1p = np.zeros((128, 5, 2, 2, 128), np.float32)
    for m in range(2):
        for kt in range(2):
            pw1p[:, 0, kt, m, :] = (A_W * S_Z / S_GB) * \
                pw1r[m * 128:(m + 1) * 128, kt].T
        for k in range(4):
            for kt in range(2):
                pw1p[:, k + 1, kt, m, :] = A_W * \
                    pw1r[m * 128:(m + 1) * 128, 2 + 2 * k + kt].T
    pb1prime = pb1 + np.einsum('ojc,jc->o', pw1r[:, 2:], C)
    pb1s = (S_H1 * pb1prime).reshape(2, 128).T.copy()

    pw2t = np.zeros((128, 2, 64), np.float32)
    for kt in range(2):
        pw2t[:, kt, :] = A_W * pw2[:, kt * 128:(kt + 1) * 128].T
    pb2s = (S_H2 * pb2).reshape(64, 1)
    pw3t = A_W * pw3.T                                # [64, 2]

    zb = (not np.any(pb0)) and (not np.any(lsb[:, :, 1])) \
        and (not np.any(pb1s)) and (not np.any(pb2s))

    # ---- pack blobs ----
    CA = 2946
    blobA = np.zeros((128, CA), F8)
    blobA[0:99, 0:1536] = _f8(w1p).reshape(99, -1)
    blobA[:, 1536:1664] = _f8(w2t).reshape(128, -1)
    blobA[0:66, 1664:2944] = _f8(headw).reshape(66, -1)
    blobA[0:64, 2944:2946] = _f8(pw3t)

    CB_RES = PADQ * 512
    CB_FUS = CB_RES + 8960
    CB_PW1 = CB_FUS + 2048
    CB_PW2 = CB_PW1 + 2560
    CB = CB_PW2 + 128
    blobB_shared = np.zeros((128, CB - CB_RES), F8)
    blobB_shared[:, 0:8960] = _f8(resw).reshape(128, -1)
    blobB_shared[:, 8960:8960 + 2048] = _f8(fusw).reshape(128, -1)
    blobB_shared[:, 11008:11008 + 2560] = _f8(pw1p).reshape(128, -1)
    pw2pad = np.zeros((128, 2, 64), np.float32)
    pw2pad[:, :, :] = pw2t[:, :, 0:64]
    blobB_shared[:, 13568:13696] = _f8(pw2pad).reshape(128, -1)

    CC = 24 + PADQ * 2
    consts_shared = np.zeros((128, CC), np.float32)
    consts_shared[:, 0:2] = pb0s
    consts_shared[:, 2:18] = lsb.reshape(128, -1)
    consts_shared[:, 18:20] = fusc
    consts_shared[:, 20:22] = pb1s
    consts_shared[0:64, 22:23] = pb2s

    shared = {"blobA": blobA}
    if with_b2:
        shared["b2r"] = _f8(S_FEAT * b2.reshape(1, 64))

    in_maps = []
    for c in range(N_CORES):
        img = cnn[c]
        img_pad = np.zeros((C_IN, PADW, PADW), np.float32)
        img_pad[:, 1:129, 1:129] = img
        flatf = _f8(img_pad.reshape(C_IN, PIMG)).astype(np.float32)
        stk = np.zeros((99, 2, PIMG), np.float32)
        for r in range(99):
            ch, kh = ROWMAP_A[r]
            ln = PIMG - kh * PADW
            stk[r, 0, :ln] = flatf[ch, kh * PADW:]
            ch, kh = ROWMAP_B[r]
            ln = PIMG - kh * PADW
            stk[r, 1, :ln] = flatf[ch, kh * PADW:]

        own = order[offs[c]:offs[c + 1]]
        nown = len(own)
        gixa = np.zeros(NV, np.int64)
        gixb = np.zeros(NV, np.int64)
        dgt = np.zeros((128, PADQ, 4, 128), np.float32)
        ar = np.arange(128)
        for qi, poly in enumerate(own):
            gixa[qi * 128:(qi + 1) * 128] = idxA[poly]
            gixb[qi * 128:(qi + 1) * 128] = idxB[poly]
            dgt[ar, qi, 0, ar] = wA0[poly]
            dgt[ar, qi, 1, ar] = wA1[poly]
            dgt[ar, qi, 2, ar] = wB0[poly]
            dgt[ar, qi, 3, ar] = wB1[poly]

        cpv = np.zeros((2, PADQ, 160), np.float32)
        if nown:
            cc = (cpoly[own] * RO).transpose(2, 0, 1)     # [2, nown, 128]
            cpv[:, :nown, 16:144] = cc
            cpv[:, :nown, 0:16] = cc[:, :, 112:128]
            cpv[:, :nown, 144:160] = cc[:, :, 0:16]

        base = np.zeros((128, PADQ, 2), np.float32)
        if nown:
            base[:, :nown, :] = (ipoly[own] * RO + pb3[None, None, :]) \
                .transpose(1, 0, 2)

        blobB = np.zeros((128, CB), F8)
        blobB[:, 0:CB_RES] = _f8(dgt).reshape(128, -1)
        blobB[:, CB_RES:] = blobB_shared
        consts = consts_shared.copy()
        consts[:, 24:] = base.reshape(128, -1)
        gix = np.concatenate([pack16(gixa, NV // 16),
                              pack16(gixb, NV // 16)], axis=1)
        m = {
            "stk": _f8(stk), "blobB": blobB, "consts": consts,
            "gix": gix, "cpv": _f8(cpv),
        }
        if with_b2:
            svr = np.zeros((1, PADQ, 128), np.float32)
            svr[0, :nown, :] = s_v[own]
            m["svr"] = _f8(svr)
        m.update(shared)
        in_maps.append(m)
    return in_maps, with_b2, zb


def kernel(**inputs):
    ind = np.asarray(inputs["ind"]).astype(np.int64)
    counts = np.bincount(ind, minlength=N_CORES)
    P = int(counts.max())
    order = np.argsort(ind, kind="stable")
    offs = np.concatenate([[0], np.cumsum(counts)])

    in_maps, with_b2, zb = _host_prep(inputs, P, counts, order, offs)
    nc = _get_nc_key(P, with_b2, zb)
    res = None
    last_err = None
    for _attempt in range(3):
        try:
            res = run_bass_kernel_spmd(nc, in_maps, list(range(N_CORES)))
            break
        except Exception as e:  # rare transient device error; retry
            last_err = e
    if res is None:
        raise last_err

    out = np.zeros((NP, V, 2), np.float32)
    for c in range(N_CORES):
        oc = res.results[c]["out"]  # [128v, PADQ, 2]
        own = order[offs[c]:offs[c + 1]]
        for q, opoly in enumerate(own):
            out[opoly] = oc[:, q, :]
    return out


# revision 14
# speedup vs baseline: 1.0675x; 1.0675x over previous
"""Trainium2 Bass kernel for nn_Evolution_26697516712465 (deep-snake GNN).

Self-contained: takes FULL inputs, shards batch across 8 NeuronCores internally
(one image per core; each core runs the snake for the polys of its own image),
returns FULL output [128, 128, 2] fp32.

fp8e4 (e4m3) DoubleRow matmuls throughout (2 contraction rows/cycle), weights
pre-scaled by 64 into fp8 normal range, activations carried at power-of-2
scales; bilinear grid-sample folded into PE "diagonal" matmuls on gathered
corner row-pairs; eval-mode bn folded into weights/biases host-side.
"""
import numpy as np
import ml_dtypes
from contextlib import ExitStack

import concourse.bass as bass
import concourse.bacc as bacc
import concourse.mybir as mybir
import concourse.tile as tile
from concourse.library_config import mlp as mlp_lib
from concourse.bass_utils import run_bass_kernel_spmd

N_CORES = 8
B, C_IN, H, W = 8, 66, 128, 128
NP, V = 128, 128
RO = 4.0
DIL = (1, 1, 1, 2, 2, 4, 4)
NRES = 7
HW = H * W          # 16384
PADW = W + 2        # 130
PIMG = PADW * PADW  # 16900
PADV = 160          # 16 + 128 + 16 circular pad

f32 = mybir.dt.float32
f32r = mybir.dt.float32r
fp8 = mybir.dt.float8e4
i16 = mybir.dt.int16
AF = mybir.ActivationFunctionType
ALU = mybir.AluOpType
DR = mybir.MatmulPerfMode.DoubleRow

F8 = ml_dtypes.float8_e4m3

# activation/weight scales (powers of 2)
A_W = 64.0          # weight scale
S_R1 = 8.0          # conv1 relu out
S_FEAT = 32.0       # ipad feat rows (folded into diag weights)
S_Z = 32.0          # snake states
S_GB = 128.0        # fusion global feature
S_H1 = 128.0
S_H2 = 512.0

# conv1 stack row maps: blkA = 66ch kh0 + 33ch kh1; blkB = 33ch kh1 + 66ch kh2
ROWMAP_A = [(r, 0) if r < 66 else (r - 66, 1) for r in range(99)]
ROWMAP_B = [(r + 33, 1) if r < 33 else (r - 33, 2) for r in range(99)]


def _f8(x):
    return np.clip(np.asarray(x, np.float32), -240.0, 240.0).astype(F8)


def pack16(idx_flat, cols):
    tab = np.zeros((16, cols), np.int16)
    n = len(idx_flat)
    tab[np.arange(n) % 16, np.arange(n) // 16] = idx_flat.astype(np.int16)
    return np.tile(tab, (8, 1))


def build_nc(P, with_b2, zb):
    """Build the SPMD Bass program. P = max polys per image; zb = all relu
    biases are zero (allows relu on DVE/Pool engines)."""
    nc = bacc.Bacc("TRN2", target_bir_lowering=False, debug=False)
    PADQ = -(-P // 4) * 4
    NQB = PADQ // 4
    NV = PADQ * 128

    # ---------------- inputs ----------------
    # blobA: w1p | w2t | headw | pw3 (fp8, conv+early weights)
    CA_W1P, CA_W2T, CA_HW, CA_PW3 = 0, 1536, 1664, 2944
    CA = 2946
    # blobB: dgt | resw | fusw | pw1 | pw2 (fp8, late weights)
    CB_DGT = 0
    CB_RES = PADQ * 512
    CB_FUS = CB_RES + 8960
    CB_PW1 = CB_FUS + 2048
    CB_PW2 = CB_PW1 + 2560
    CB = CB_PW2 + 128
    # consts (f32): pb0 | lsb | fusc | pb1 | pb2 | base
    CC_PB0, CC_LSB, CC_FUSC, CC_PB1, CC_PB2, CC_BASE = 0, 2, 18, 20, 22, 24
    CC = 24 + PADQ * 2
    d_stk = nc.declare_dram_parameter("stk", [99, 2 * PIMG], fp8, isOutput=False)
    d_blobA = nc.declare_dram_parameter("blobA", [128, CA], fp8, isOutput=False)
    d_blobB = nc.declare_dram_parameter("blobB", [128, CB], fp8, isOutput=False)
    d_consts = nc.declare_dram_parameter("consts", [128, CC], f32, isOutput=False)
    d_gix = nc.declare_dram_parameter("gix", [128, 2 * (NV // 16)], i16, isOutput=False)
    d_cpv = nc.declare_dram_parameter("cpv", [2, PADQ, 160], fp8, isOutput=False)
    if with_b2:
        d_b2r = nc.declare_dram_parameter("b2r", [1, 64], fp8, isOutput=False)
        d_svr = nc.declare_dram_parameter("svr", [1, PADQ, 128], fp8, isOutput=False)
    d_out = nc.declare_dram_parameter("out", [128, PADQ, 2], f32, isOutput=True)

    feat_dram = nc.dram_tensor("feat_dram", [HW, 64], f32)

    with tile.TileContext(nc, num_cores=N_CORES) as tc, ExitStack() as top:
        wpool = top.enter_context(tc.tile_pool(name="weights", bufs=1))
        # SP queue: blobA, consts, gix (small, early)
        blobA = wpool.tile([128, CA], fp8)
        nc.sync.dma_start(out=blobA, in_=d_blobA[:, :])
        consts = wpool.tile([128, CC], f32)
        nc.sync.dma_start(out=consts, in_=d_consts[:, :])
        gix_t = wpool.tile([128, 2 * (NV // 16)], i16)
        nc.sync.dma_start(out=gix_t, in_=d_gix[:, :])
        if with_b2:
            b2r_t = wpool.tile([1, 64], fp8)
            nc.sync.dma_start(out=b2r_t, in_=d_b2r[:, :])
            svr_t = wpool.tile([1, PADQ, 128], fp8)
            nc.sync.dma_start(out=svr_t, in_=d_svr[:, :, :])

        nc.gpsimd.load_library(mlp_lib)

        # Pool DMA queue: stk bands, then blobB chunks (dgt first)
        stk_t = wpool.tile([99, 2 * PIMG], fp8)
        CHK = 2 * 16 * PADW
        for bb in range(8):
            nc.sync.dma_start(out=stk_t[:, bb * CHK:(bb + 1) * CHK],
                              in_=d_stk[:, bb * CHK:(bb + 1) * CHK])
        blobB = wpool.tile([128, CB], fp8)
        nc.gpsimd.dma_start(out=blobB[:, CB_DGT:CB_RES],
                            in_=d_blobB[:, CB_DGT:CB_RES])
        for i in range(NRES):
            c0 = CB_RES + i * 1280
            nc.gpsimd.dma_start(out=blobB[:, c0:c0 + 1280],
                                in_=d_blobB[:, c0:c0 + 1280])
        nc.gpsimd.dma_start(out=blobB[:, CB_FUS:CB],
                            in_=d_blobB[:, CB_FUS:CB])

        aps = blobA.ap[0][0]
        bps = blobB.ap[0][0]
        cps = consts.ap[0][0]

        def ap_of(blob, pstride, parts, col0, dims):
            return bass.AP(tensor=blob.tensor, offset=blob.offset + col0,
                           ap=[[pstride, parts]] + dims)

        def w1p_ap(p, m):
            return ap_of(blobA, aps, 99, CA_W1P + p * 512 + m * 128,
                         [[256, 2], [1, 128]])

        def w2t_ap():
            return ap_of(blobA, aps, 128, CA_W2T, [[64, 2], [1, 64]])

        def headw_ap(p):
            return ap_of(blobA, aps, 66, CA_HW + p * 256, [[128, 2], [1, 128]])

        def pw3_ap():
            return ap_of(blobA, aps, 64, CA_PW3, [[1, 2]])

        def dgt_ap(q, gi):
            return ap_of(blobB, bps, 128, CB_DGT + q * 512 + gi * 256,
                         [[128, 2], [1, 128]])

        def resw_ap(i, p, m):
            return ap_of(blobB, bps, 128, CB_RES + i * 1280 + p * 256,
                         [[128, 2], [1, 128]])

        def fusw_ap(k, m):
            return ap_of(blobB, bps, 128, CB_FUS + k * 512 + m * 128,
                         [[256, 2], [1, 128]])

        def pw1_ap(p, m):
            return ap_of(blobB, bps, 128, CB_PW1 + p * 512 + m * 128,
                         [[256, 2], [1, 128]])

        def pw2_ap():
            return ap_of(blobB, bps, 128, CB_PW2, [[64, 2], [1, 64]])

        def const_col(c0, parts=128, n=1):
            return ap_of(consts, cps, parts, c0, [[1, n]])

        base_ap = ap_of(consts, cps, 128, CC_BASE, [[2, PADQ], [1, 2]])

        # --- engine helpers ---
        rr_state = [0]

        def rr_relu(out_ap, in_ap, scale, bias_ap, force=None):
            """out = relu(scale*x + bias). scale may be const or AP.
            If zb (bias==0), can run on act or dve; else activation only.
            (GPSIMD cannot access PSUM, so pool never does these.)"""
            if not zb or force == 'act':
                nc.scalar.activation(out_ap, in_ap, AF.Relu,
                                     bias=(bias_ap if (bias_ap is not None and
                                                       not zb) else 0.0),
                                     scale=scale)
                return
            e = force if force is not None else ('act', 'dve')[rr_state[0] % 2]
            if force is None:
                rr_state[0] += 1
            if e == 'act':
                nc.scalar.activation(out_ap, in_ap, AF.Relu, bias=0.0,
                                     scale=scale)
            else:
                nc.vector.tensor_scalar(out_ap, in_ap, scale, 0.0,
                                        op0=ALU.mult, op1=ALU.max)

        def rr_copy(out_ap, in_ap, scale=None, force=None, pool_ok=False):
            engs = ('act', 'dve', 'pool') if pool_ok else ('act', 'dve')
            e = force if force is not None else engs[rr_state[0] % len(engs)]
            if force is None:
                rr_state[0] += 1
            if scale is None:
                if e == 'act':
                    nc.scalar.activation(out_ap, in_ap, AF.Copy, bias=0.0)
                elif e == 'dve':
                    nc.vector.tensor_copy(out_ap, in_ap)
                else:
                    nc.gpsimd.tensor_copy(out_ap, in_ap)
            else:
                if e == 'act':
                    nc.scalar.activation(out_ap, in_ap, AF.Copy, bias=0.0,
                                         scale=scale)
                elif e == 'dve':
                    nc.vector.tensor_scalar(out_ap, in_ap, scale, None,
                                            op0=ALU.mult)
                else:
                    nc.gpsimd.tensor_scalar(out_ap, in_ap, scale, None,
                                            op0=ALU.mult)

        # ------------ conv1 (3x3, 66->256) + conv2 (1x1, 256->64) ------------
        with tc.tile_pool(name="conv", bufs=1) as cpool, \
             tc.tile_pool(name="psumA", bufs=2, space="PSUM") as ppA, \
             tc.tile_pool(name="psumB", bufs=2, space="PSUM") as ppB, \
             tc.tile_pool(name="stage", bufs=3) as spool:
            r1 = cpool.tile([128, 2, HW], fp8)
            ps2 = {}

            def emit_conv2(g):
                h = g % 2
                if h == 0:
                    ps2[0] = ppB.tile([128, 8, 64], f32, tag="c2", name="c2")
                for cc in range(4):
                    pos0 = g * 512 + cc * 128
                    lhsT = bass.AP(tensor=r1.tensor, offset=r1.offset + pos0,
                                   ap=[r1.ap[0], [HW, 2], [1, 128]])
                    nc.tensor.matmul(ps2[0][:, h * 4 + cc, :], lhsT, w2t_ap(),
                                     start=True, stop=True, perf_mode=DR)
                if h == 1:
                    stg = spool.tile([128, 8, 64], f32, tag="stage", name="stg")
                    rr_copy(stg, ps2[0], 1.0 / (A_W * S_R1))
                    dst = bass.AP(tensor=feat_dram,
                                  offset=((g - 1) * 512) * 64,
                                  ap=[[64, 128], [8192, 8], [1, 64]])
                    nc.sync.dma_start(out=dst, in_=stg)

            for g in range(32):              # y-groups of 4 rows
                for m in range(2):
                    pg = ppA.tile([128, 4, 128], f32, tag=f"c1_{m}",
                                  name=f"c1_{m}")
                    for yy in range(4):
                        y = 4 * g + yy
                        ktaps = ((2 * y * PADW, 2), (2 * y * PADW + 1, 3),
                                 (2 * (y * PADW + 1) + 1, 2))
                        for p, (off, stride) in enumerate(ktaps):
                            rhs = bass.AP(tensor=stk_t.tensor,
                                          offset=stk_t.offset + off,
                                          ap=[stk_t.ap[0], [stride, 2],
                                              [2, 128]])
                            nc.tensor.matmul(pg[:, yy, :], w1p_ap(p, m),
                                             rhs, start=(p == 0), stop=(p == 2),
                                             perf_mode=DR)
                    rr_relu(r1[:, m, g * 512:(g + 1) * 512],
                            pg.rearrange("p a b -> p (a b)"), S_R1 / A_W,
                            const_col(CC_PB0 + m))
                if g >= 2:
                    emit_conv2(g - 2)        # skewed to avoid PE queue stall
            emit_conv2(30)
            emit_conv2(31)

        # ------------ gather + combine-transpose into ipad ------------
        with tc.tile_pool(name="snake", bufs=1) as sn:
            ipad = sn.tile([128, PADQ, PADV], fp8, tag="ipad", name="ipad")
            zall = sn.tile([128, 8, PADQ, PADV], fp8, tag="zall", name="zall")
            rsc = sn.tile([128, PADQ, 128], fp8, tag="rsc", name="rsc")

            with tc.tile_pool(name="gat", bufs=1) as gp, \
                 tc.tile_pool(name="psumG", bufs=6, space="PSUM") as ppG:
                gta = gp.tile([128, PADQ, 128], f32, tag="gta", name="gta")
                gtb = gp.tile([128, PADQ, 128], f32, tag="gtb", name="gtb")
                gsrc = bass.AP(tensor=feat_dram, offset=0,
                               ap=[[64, HW - 1], [1, 128]])
                nc.gpsimd.dma_gather(gta, gsrc, gix_t[:, 0:NV // 16], NV, NV,
                                     128, elem_step=64, single_packet=False)
                nc.gpsimd.dma_gather(gtb, gsrc, gix_t[:, NV // 16:], NV, NV,
                                     128, elem_step=64, single_packet=False)
                nc.sync.dma_start(out=ipad[64:66, :, :], in_=d_cpv[:, :, :])

                # convert gathered corners f32 -> fp8 (x S_FEAT)
                g8a = gp.tile([128, PADQ, 128], fp8, tag="g8a", name="g8a")
                g8b = gp.tile([128, PADQ, 128], fp8, tag="g8b", name="g8b")
                for src, dst in ((gta, g8a), (gtb, g8b)):
                    for qb in range(NQB):
                        qsl = slice(4 * qb, 4 * qb + 4)
                        rr_copy(dst[:, qsl, :], src[:, qsl, :],
                                scale=S_FEAT, pool_ok=True)

                for qb in range(NQB):
                    pg = ppG.tile([64, 4, 128], f32, tag="dg", name="dg")
                    for qq in range(4):
                        q = 4 * qb + qq
                        last = 2 if with_b2 else 1
                        for gi, gt in enumerate((g8a, g8b)):
                            lhsT = bass.AP(
                                tensor=gt.tensor,
                                offset=gt.offset + q * 128,
                                ap=[gt.ap[0], [64, 2], [1, 64]])
                            nc.tensor.matmul(pg[:, qq, :], lhsT,
                                             dgt_ap(q, gi),
                                             start=(gi == 0), stop=(gi == last),
                                             perf_mode=DR)
                        if with_b2:
                            nc.tensor.matmul(pg[:, qq, :], b2r_t[:, :],
                                             svr_t[:, q, :], start=False,
                                             stop=True)
                    rr_copy(ipad[0:64, 4 * qb:4 * qb + 4, 16:144], pg)
                nc.vector.tensor_copy(ipad[0:66, :, 0:16],
                                      ipad[0:66, :, 128:144])
                nc.gpsimd.tensor_copy(ipad[0:66, :, 144:160],
                                      ipad[0:66, :, 16:32])

            # ------------ snake ------------
            with tc.tile_pool(name="psumS", bufs=4, space="PSUM") as ppS:

                def conv_layer(zo, rhs_base_fn, rhs_tensor, rhs_ap0, lhsT_fn,
                               dil, src_zi):
                    """One circular conv layer; per-qb relu (+residual add)."""
                    for qb in range(NQB):
                        ps = ppS.tile([128, 4, 128], f32, tag="psS", name="psS")
                        for qq in range(4):
                            q = 4 * qb + qq
                            for p in range(5):
                                if p < 4:
                                    off = rhs_base_fn(q) + 16 + (2 * p - 4) * dil
                                    stride = dil
                                else:
                                    off = rhs_base_fn(q) + 16 + 4 * dil
                                    stride = 0
                                rhs = bass.AP(tensor=rhs_tensor, offset=off,
                                              ap=[rhs_ap0, [stride, 2],
                                                  [1, 128]])
                                nc.tensor.matmul(ps[:, qq, :], lhsT_fn(p), rhs,
                                                 start=(p == 0), stop=(p == 4),
                                                 perf_mode=DR)
                        qsl = slice(4 * qb, 4 * qb + 4)
                        scale_ap = const_col(CC_LSB + 2 * zo)
                        bias_ap = const_col(CC_LSB + 2 * zo + 1)
                        if src_zi is None:
                            # head: write z0 directly
                            rr_relu(zall[:, 0, qsl, 16:144], ps, scale_ap,
                                    bias_ap, force=('act' if qb != 4 else 'dve'))
                        else:
                            rr_relu(rsc[:, qsl, :], ps, scale_ap, bias_ap,
                                    force=('act' if qb != 4 else 'dve'))
                            addeng = nc.gpsimd if qb in (1, 2) else nc.vector
                            addeng.tensor_tensor(
                                zall[:, zo, qsl, 16:144],
                                zall[:, src_zi, qsl, 16:144],
                                rsc[:, qsl, :], ALU.add)
                        if qb == 0:
                            # urgent wraps so next layer's qb0 can start
                            nc.vector.tensor_copy(zall[:, zo, 0:4, 0:16],
                                                  zall[:, zo, 0:4, 128:144])
                            nc.vector.tensor_copy(zall[:, zo, 0:4, 144:160],
                                                  zall[:, zo, 0:4, 16:32])
                    nc.vector.tensor_copy(zall[:, zo, 4:PADQ, 0:16],
                                          zall[:, zo, 4:PADQ, 128:144])
                    nc.gpsimd.tensor_copy(zall[:, zo, 4:PADQ, 144:160],
                                          zall[:, zo, 4:PADQ, 16:32])

                ip66 = ipad[0:66, :, :]
                conv_layer(0, lambda q: ip66.offset + q * PADV, ip66.tensor,
                           ip66.ap[0], headw_ap, 1, None)
                for i in range(NRES):
                    zi_off = zall.offset + i * PADQ * PADV
                    conv_layer(i + 1,
                               lambda q, zi_off=zi_off: zi_off + q * PADV,
                               zall.tensor, zall.ap[0],
                               lambda p, i=i: resw_ap(i, p, 0),
                               DIL[i], i)

                # fusion 1x1 (1024->256) + per-poly max over V
                gmax = sn.tile([128, 2, PADQ], f32, tag="gmax", name="gmax")
                gb = sn.tile([128, 2, PADQ], fp8, tag="gb", name="gb")
                for m in range(2):
                    for qb in range(NQB):
                        ps = ppS.tile([128, 4, 128], f32, tag="psS", name="psS")
                        for qq in range(4):
                            q = 4 * qb + qq
                            for k in range(4):
                                off = zall.offset + (2 * k * PADQ + q) * PADV + 16
                                rhs = bass.AP(tensor=zall.tensor, offset=off,
                                              ap=[zall.ap[0],
                                                  [PADQ * PADV, 2], [1, 128]])
                                nc.tensor.matmul(ps[:, qq, :],
                                                 fusw_ap(k, m), rhs,
                                                 start=(k == 0), stop=(k == 3),
                                                 perf_mode=DR)
                        nc.vector.tensor_reduce(gmax[:, m, 4 * qb:4 * qb + 4],
                                                ps, axis=mybir.AxisListType.X,
                                                op=ALU.max)
                    nc.vector.tensor_scalar(gb[:, m, :], gmax[:, m, :],
                                            S_GB / (A_W * S_Z),
                                            const_col(CC_FUSC + m),
                                            op0=ALU.mult, op1=ALU.add)

                # pred1: 1280 -> 256 relu
                h1 = sn.tile([128, 2, PADQ, 128], fp8, tag="h1", name="h1")
                for m in range(2):
                    for qb in range(NQB):
                        ps = ppS.tile([128, 4, 128], f32, tag="psS", name="psS")
                        for qq in range(4):
                            q = 4 * qb + qq
                            for k in range(4):
                                off = zall.offset + (2 * k * PADQ + q) * PADV + 16
                                rhs = bass.AP(tensor=zall.tensor, offset=off,
                                              ap=[zall.ap[0],
                                                  [PADQ * PADV, 2], [1, 128]])
                                nc.tensor.matmul(ps[:, qq, :],
                                                 pw1_ap(k + 1, m), rhs,
                                                 start=(k == 0), stop=False,
                                                 perf_mode=DR)
                            rhs0 = bass.AP(tensor=gb.tensor,
                                           offset=gb.offset + q,
                                           ap=[gb.ap[0], [PADQ, 2], [0, 128]])
                            nc.tensor.matmul(ps[:, qq, :], pw1_ap(0, m),
                                             rhs0, start=False, stop=True,
                                             perf_mode=DR)
                        rr_relu(h1[:, m, 4 * qb:4 * qb + 4, :], ps,
                                S_H1 / (A_W * S_Z), const_col(CC_PB1 + m))

            # pred2 + pred3
            with tc.tile_pool(name="psumT", bufs=3, space="PSUM") as ppT:
                h2 = sn.tile([64, PADQ, 128], fp8, tag="h2", name="h2")
                for qb in range(NQB):
                    ps = ppT.tile([64, 4, 128], f32, tag="psT", name="psT")
                    for qq in range(4):
                        q = 4 * qb + qq
                        rhs = bass.AP(tensor=h1.tensor,
                                      offset=h1.offset + q * 128,
                                      ap=[h1.ap[0], [PADQ * 128, 2], [1, 128]])
                        nc.tensor.matmul(ps[:, qq, :], pw2_ap(), rhs,
                                         start=True, stop=True, perf_mode=DR)
                    rr_relu(h2[:, 4 * qb:4 * qb + 4, :], ps,
                            S_H2 / (A_W * S_H1), const_col(CC_PB2, parts=64))

                ps3 = ppT.tile([128, PADQ, 2], f32, tag="psT3", name="psT3",
                               bufs=1)
                for q in range(PADQ):
                    nc.tensor.matmul(ps3[:, q, :], h2[:, q, :], pw3_ap(),
                                     start=True, stop=True)
                o_f = sn.tile([128, PADQ, 2], f32, tag="o_f", name="o_f")
                nc.vector.tensor_scalar(o_f, ps3, 1.0 / (A_W * S_H2), None,
                                        op0=ALU.mult)
                o_t = sn.tile([128, PADQ, 2], f32, tag="o_t", name="o_t")
                nc.vector.tensor_tensor(o_t, o_f, base_ap, ALU.add)
                nc.sync.dma_start(out=d_out[:, :, :], in_=o_t)

    nc.compile()
    return nc


_NC_CACHE = {}


def _get_nc_key(P, with_b2, zb):
    key = (P, with_b2, zb)
    if key not in _NC_CACHE:
        _NC_CACHE[key] = build_nc(P, with_b2, zb)
    return _NC_CACHE[key]


def _get_nc(P):
    """test.py compatibility: default flags for the standard input set."""
    return _get_nc_key(P, False, True)


def _host_prep(inputs, P, counts, order, offs):
    """Build per-core in_maps. Returns (in_maps, with_b2, zb)."""
    PADQ = -(-P // 4) * 4
    NV = PADQ * 128
    cnn = np.asarray(inputs["cnn_feature"], np.float32)
    ipoly = np.asarray(inputs["i_it_poly"], np.float32)
    cpoly = np.asarray(inputs["c_it_poly"], np.float32)
    w1 = np.asarray(inputs["proj_w1"], np.float32)
    pb0 = np.asarray(inputs["proj_b1"], np.float32)
    w2 = np.asarray(inputs["proj_w2"], np.float32)[:, :, 0, 0]  # [64, 256]
    b2 = np.asarray(inputs["proj_b2"], np.float32)
    head_w = np.asarray(inputs["head_w"], np.float32)   # [128, 66, 9]
    head_b = np.asarray(inputs["head_b"], np.float32)
    head_g = np.asarray(inputs["head_g"], np.float32)
    head_bt = np.asarray(inputs["head_bt"], np.float32)
    res_w = np.asarray(inputs["res_w"], np.float32)     # [7, 128, 128, 9]
    res_b = np.asarray(inputs["res_b"], np.float32)
    res_g = np.asarray(inputs["res_g"], np.float32)
    res_bt = np.asarray(inputs["res_bt"], np.float32)
    fus_w = np.asarray(inputs["fus_w"], np.float32)     # [256, 1024]
    fus_b = np.asarray(inputs["fus_b"], np.float32)
    pw1 = np.asarray(inputs["pw1"], np.float32)         # [256, 1280]
    pb1 = np.asarray(inputs["pb1"], np.float32)
    pw2 = np.asarray(inputs["pw2"], np.float32)         # [64, 256]
    pb2 = np.asarray(inputs["pb2"], np.float32)
    pw3 = np.asarray(inputs["pw3"], np.float32)         # [2, 64]
    pb3 = np.asarray(inputs["pb3"], np.float32)

    assert (head_g > 0).all() and (res_g > 0).all(), "bn fold requires g>0"
    with_b2 = bool(np.any(b2 != 0))

    # w1p [99, 3pairs, 2kt, 2m, 128]
    w1p = np.zeros((99, 3, 2, 2, 128), np.float32)
    pair_src = [((0, 0), (0, 1)), ((1, 0), (0, 2)), ((1, 1), (1, 2))]
    for p, pr in enumerate(pair_src):
        for kt, (blk, kw) in enumerate(pr):
            rm = ROWMAP_A if blk == 0 else ROWMAP_B
            for r in range(99):
                ch, kh = rm[r]
                for m in range(2):
                    w1p[r, p, kt, m, :] = A_W * w1[m * 128:(m + 1) * 128,
                                                   ch, kh, kw]
    pb0s = (S_R1 * pb0).reshape(2, 128).T.copy()

    w2t = np.zeros((128, 2, 64), np.float32)
    for kt in range(2):
        w2t[:, kt, :] = A_W * w2[:, kt * 128:(kt + 1) * 128].T

    # ---- grid-sample host math ----
    ix = ipoly[..., 0] - np.float32(0.5)
    iy = ipoly[..., 1] - np.float32(0.5)
    x0 = np.floor(ix); y0 = np.floor(iy)
    wx = (ix - x0).astype(np.float32); wy = (iy - y0).astype(np.float32)
    x0i = x0.astype(np.int64); y0i = y0.astype(np.int64)

    swap_x = x0i < 0
    vx0 = (x0i >= 0) & (x0i < W)
    vx1 = (x0i + 1 >= 0) & (x0i + 1 < W)

    def slot_weights(yi):
        vy = (yi >= 0) & (yi < H)
        w_s0 = (1 - wx) * vx0 * vy
        w_s1 = wx * vx1 * vy
        w_s0 = np.where(swap_x, wx * vx1 * vy, w_s0)
        w_s1 = np.where(swap_x, 0.0, w_s1)
        return w_s0.astype(np.float32), w_s1.astype(np.float32)

    x0c = np.clip(x0i, 0, W - 2)
    y0c = np.clip(y0i, 0, H - 1)
    y1c = np.clip(y0i + 1, 0, H - 1)
    idxA = (y0c * W + x0c).astype(np.int64)          # [NP, V]
    idxB = (y1c * W + x0c).astype(np.int64)
    wA0, wA1 = slot_weights(y0i)
    wB0, wB1 = slot_weights(y0i + 1)
    wA0 *= (1 - wy); wA1 *= (1 - wy)
    wB0 *= wy; wB1 *= wy
    s_v = wA0 + wA1 + wB0 + wB1

    # ---- snake weights (bn + scale folds) ----
    headw = np.zeros((66, 5, 2, 128), np.float32)
    hw9 = head_w.transpose(1, 2, 0)                  # [66, 9, 128]
    for t in range(9):
        p, kt = t // 2, t % 2
        headw[0:64, p, kt, :] = A_W * hw9[0:64, t, :]
        headw[64:66, p, kt, :] = A_W * S_FEAT * hw9[64:66, t, :]
    lsb = np.zeros((128, 8, 2), np.float32)
    lsb[:, 0, 0] = head_g / A_W
    lsb[:, 0, 1] = S_Z * head_g * head_b
    C = np.zeros((8, 128), np.float32)               # C_i = sum_{j<=i} bt_j
    C[0] = head_bt
    for i in range(NRES):
        C[i + 1] = C[i] + res_bt[i]

    resw = np.zeros((128, 7, 5, 2, 128), np.float32)
    for i in range(NRES):
        rw = res_w[i].transpose(1, 2, 0)             # [128 in, 9, 128 out]
        for t in range(9):
            p, kt = t // 2, t % 2
            resw[:, i, p, kt, :] = A_W * rw[:, t, :]
        bprime = res_b[i] + res_w[i].sum(axis=2) @ C[i]
        lsb[:, i + 1, 0] = res_g[i] / A_W
        lsb[:, i + 1, 1] = S_Z * res_g[i] * bprime

    fw8 = fus_w.reshape(256, 8, 128)
    fusw = np.zeros((128, 4, 2, 2, 128), np.float32)
    for k in range(4):
        for kt in range(2):
            for m in range(2):
                fusw[:, k, kt, m, :] = A_W * fw8[m * 128:(m + 1) * 128,
                                                 2 * k + kt].T
    fusconst = fus_b + np.einsum('ojc,jc->o', fw8, C)
    fusc = (S_GB * fusconst).reshape(2, 128).T.copy()

    pw1r = pw1.reshape(256, 10, 128)
    pw1p = np.zeros((128, 5, 2, 2, 128), np.float32)
    for m in range(2):
        for kt in range(2):
            pw1p[:, 0, kt, m, :] = (A_W * S_Z / S_GB) * \
                pw1r[m * 128:(m + 1) * 128, kt].T
        for k in range(4):
            for kt in range(2):
                pw1p[:, k + 1, kt, m, :] = A_W * \
                    pw1r[m * 128:(m + 1) * 128, 2 + 2 * k + kt].T
    pb1prime = pb1 + np.einsum('ojc,jc->o', pw1r[:, 2:], C)
    pb1s = (S_H1 * pb1prime).reshape(2, 128).T.copy()

    pw2t = np.zeros((128, 2, 64), np.float32)
    for kt in range(2):
        pw2t[:, kt, :] = A_W * pw2[:, kt * 128:(kt + 1) * 128].T
    pb2s = (S_H2 * pb2).reshape(64, 1)
    pw3t = A_W * pw3.T                                # [64, 2]

    zb = (not np.any(pb0)) and (not np.any(lsb[:, :, 1])) \
        and (not np.any(pb1s)) and (not np.any(pb2s))

    # ---- pack blobs ----
    CA = 2946
    blobA = np.zeros((128, CA), F8)
    blobA[0:99, 0:1536] = _f8(w1p).reshape(99, -1)
    blobA[:, 1536:1664] = _f8(w2t).reshape(128, -1)
    blobA[0:66, 1664:2944] = _f8(headw).reshape(66, -1)
    blobA[0:64, 2944:2946] = _f8(pw3t)

    CB_RES = PADQ * 512
    CB_FUS = CB_RES + 8960
    CB_PW1 = CB_FUS + 2048
    CB_PW2 = CB_PW1 + 2560
    CB = CB_PW2 + 128
    blobB_shared = np.zeros((128, CB - CB_RES), F8)
    blobB_shared[:, 0:8960] = _f8(resw).reshape(128, -1)
    blobB_shared[:, 8960:8960 + 2048] = _f8(fusw).reshape(128, -1)
    blobB_shared[:, 11008:11008 + 2560] = _f8(pw1p).reshape(128, -1)
    pw2pad = np.zeros((128, 2, 64), np.float32)
    pw2pad[:, :, :] = pw2t[:, :, 0:64]
    blobB_shared[:, 13568:13696] = _f8(pw2pad).reshape(128, -1)

    CC = 24 + PADQ * 2
    consts_shared = np.zeros((128, CC), np.float32)
    consts_shared[:, 0:2] = pb0s
    consts_shared[:, 2:18] = lsb.reshape(128, -1)
    consts_shared[:, 18:20] = fusc
    consts_shared[:, 20:22] = pb1s
    consts_shared[0:64, 22:23] = pb2s

    shared = {"blobA": blobA}
    if with_b2:
        shared["b2r"] = _f8(S_FEAT * b2.reshape(1, 64))

    in_maps = []
    for c in range(N_CORES):
        img = cnn[c]
        img_pad = np.zeros((C_IN, PADW, PADW), np.float32)
        img_pad[:, 1:129, 1:129] = img
        flatf = _f8(img_pad.reshape(C_IN, PIMG)).astype(np.float32)
        stk = np.zeros((99, PIMG, 2), np.float32)
        for r in range(99):
            ch, kh = ROWMAP_A[r]
            ln = PIMG - kh * PADW
            stk[r, :ln, 0] = flatf[ch, kh * PADW:]
            ch, kh = ROWMAP_B[r]
            ln = PIMG - kh * PADW
            stk[r, :ln, 1] = flatf[ch, kh * PADW:]
        stk = stk.reshape(99, 2 * PIMG)

        own = order[offs[c]:offs[c + 1]]
        nown = len(own)
        gixa = np.zeros(NV, np.int64)
        gixb = np.zeros(NV, np.int64)
        dgt = np.zeros((128, PADQ, 4, 128), np.float32)
        ar = np.arange(128)
        for qi, poly in enumerate(own):
            gixa[qi * 128:(qi + 1) * 128] = idxA[poly]
            gixb[qi * 128:(qi + 1) * 128] = idxB[poly]
            dgt[ar, qi, 0, ar] = wA0[poly]
            dgt[ar, qi, 1, ar] = wA1[poly]
            dgt[ar, qi, 2, ar] = wB0[poly]
            dgt[ar, qi, 3, ar] = wB1[poly]

        cpv = np.zeros((2, PADQ, 160), np.float32)
        if nown:
            cc = (cpoly[own] * RO).transpose(2, 0, 1)     # [2, nown, 128]
            cpv[:, :nown, 16:144] = cc
            cpv[:, :nown, 0:16] = cc[:, :, 112:128]
            cpv[:, :nown, 144:160] = cc[:, :, 0:16]

        base = np.zeros((128, PADQ, 2), np.float32)
        if nown:
            base[:, :nown, :] = (ipoly[own] * RO + pb3[None, None, :]) \
                .transpose(1, 0, 2)

        blobB = np.zeros((128, CB), F8)
        blobB[:, 0:CB_RES] = _f8(dgt).reshape(128, -1)
        blobB[:, CB_RES:] = blobB_shared
        consts = consts_shared.copy()
        consts[:, 24:] = base.reshape(128, -1)
        gix = np.concatenate([pack16(gixa, NV // 16),
                              pack16(gixb, NV // 16)], axis=1)
        m = {
            "stk": _f8(stk), "blobB": blobB, "consts": consts,
            "gix": gix, "cpv": _f8(cpv),
        }
        if with_b2:
            svr = np.zeros((1, PADQ, 128), np.float32)
            svr[0, :nown, :] = s_v[own]
            m["svr"] = _f8(svr)
        m.update(shared)
        in_maps.append(m)
    return in_maps, with_b2, zb


def kernel(**inputs):
    ind = np.asarray(inputs["ind"]).astype(np.int64)
    counts = np.bincount(ind, minlength=N_CORES)
    P = int(counts.max())
    order = np.argsort(ind, kind="stable")
    offs = np.concatenate([[0], np.cumsum(counts)])

    in_maps, with_b2, zb = _host_prep(inputs, P, counts, order, offs)
    nc = _get_nc_key(P, with_b2, zb)
    res = None
    last_err = None
    for _attempt in range(3):
        try:
            res = run_bass_kernel_spmd(nc, in_maps, list(range(N_CORES)))
            break
        except Exception as e:  # rare transient device error; retry
            last_err = e
    if res is None:
        raise last_err

    out = np.zeros((NP, V, 2), np.float32)
    for c in range(N_CORES):
        oc = res.results[c]["out"]  # [128v, PADQ, 2]
        own = order[offs[c]:offs[c + 1]]
        for q, opoly in enumerate(own):
            out[opoly] = oc[:, q, :]
    return out


# revision 16
# speedup vs baseline: 1.2378x; 1.1595x over previous
"""Trainium2 Bass kernel for nn_Evolution_26697516712465 (deep-snake GNN).

Self-contained: takes FULL inputs, shards batch across 8 NeuronCores internally
(one image per core; each core runs the snake for the polys of its own image),
returns FULL output [128, 128, 2] fp32.

fp8e4 (e4m3) DoubleRow matmuls throughout (2 contraction rows/cycle), weights
pre-scaled by 64 into fp8 normal range, activations carried at power-of-2
scales; bilinear grid-sample folded into PE "diagonal" matmuls on gathered
corner row-pairs; eval-mode bn folded into weights/biases host-side.
"""
import numpy as np
import ml_dtypes
from contextlib import ExitStack

import concourse.bass as bass
import concourse.bacc as bacc
import concourse.mybir as mybir
import concourse.tile as tile
from concourse.library_config import mlp as mlp_lib
from concourse.bass_utils import run_bass_kernel_spmd

N_CORES = 8
B, C_IN, H, W = 8, 66, 128, 128
NP, V = 128, 128
RO = 4.0
DIL = (1, 1, 1, 2, 2, 4, 4)
NRES = 7
HW = H * W          # 16384
PADW = W + 2        # 130
PIMG = PADW * PADW  # 16900
PADV = 160          # 16 + 128 + 16 circular pad

f32 = mybir.dt.float32
f32r = mybir.dt.float32r
fp8 = mybir.dt.float8e4
i16 = mybir.dt.int16
AF = mybir.ActivationFunctionType
ALU = mybir.AluOpType
DR = mybir.MatmulPerfMode.DoubleRow

F8 = ml_dtypes.float8_e4m3

# activation/weight scales (powers of 2)
A_W = 64.0          # weight scale
S_R1 = 8.0          # conv1 relu out
S_FEAT = 32.0       # ipad feat rows (folded into diag weights)
S_Z = 32.0          # snake states
S_GB = 128.0        # fusion global feature
S_H1 = 128.0
S_H2 = 512.0

# conv1 stack row maps: blkA = 66ch kh0 + 33ch kh1; blkB = 33ch kh1 + 66ch kh2
ROWMAP_A = [(r, 0) if r < 66 else (r - 66, 1) for r in range(99)]
ROWMAP_B = [(r + 33, 1) if r < 33 else (r - 33, 2) for r in range(99)]


def _f8(x):
    return np.clip(np.asarray(x, np.float32), -240.0, 240.0).astype(F8)


def pack16(idx_flat, cols):
    tab = np.zeros((16, cols), np.int16)
    n = len(idx_flat)
    tab[np.arange(n) % 16, np.arange(n) // 16] = idx_flat.astype(np.int16)
    return np.tile(tab, (8, 1))


def build_nc(P, with_b2, zb):
    """Build the SPMD Bass program. P = max polys per image; zb = all relu
    biases are zero (allows relu on DVE/Pool engines)."""
    nc = bacc.Bacc("TRN2", target_bir_lowering=False, debug=False)
    PADQ = -(-P // 4) * 4
    NQB = PADQ // 4
    NV = PADQ * 128

    # ---------------- inputs ----------------
    # blobA: w1p | w2t | headw | pw3 (fp8, conv+early weights)
    CA_W1P, CA_W2T, CA_HW, CA_PW3 = 0, 1536, 1664, 2944
    CA = 2946
    # blobB: dgt | resw | fusw | pw1 | pw2 (fp8, late weights)
    CB_DGT = 0
    CB_RES = PADQ * 512
    CB_FUS = CB_RES + 8960
    CB_PW1 = CB_FUS + 2048
    CB_PW2 = CB_PW1 + 2560
    CB = CB_PW2 + 128
    # consts (f32): pb0 | lsb | fusc | pb1 | pb2 | base
    CC_PB0, CC_LSB, CC_FUSC, CC_PB1, CC_PB2, CC_BASE = 0, 2, 18, 20, 22, 24
    CC = 24 + PADQ * 2
    d_stk = nc.declare_dram_parameter("stk", [99, 2 * PIMG], fp8, isOutput=False)
    d_blobA = nc.declare_dram_parameter("blobA", [128, CA], fp8, isOutput=False)
    d_blobB = nc.declare_dram_parameter("blobB", [128, CB], fp8, isOutput=False)
    d_consts = nc.declare_dram_parameter("consts", [128, CC], f32, isOutput=False)
    d_gix = nc.declare_dram_parameter("gix", [128, 2 * (NV // 16)], i16, isOutput=False)
    d_cpv = nc.declare_dram_parameter("cpv", [2, PADQ, 160], fp8, isOutput=False)
    if with_b2:
        d_b2r = nc.declare_dram_parameter("b2r", [1, 64], fp8, isOutput=False)
        d_svr = nc.declare_dram_parameter("svr", [1, PADQ, 128], fp8, isOutput=False)
    d_out = nc.declare_dram_parameter("out", [128, PADQ, 2], f32, isOutput=True)

    feat_dram = nc.dram_tensor("feat_dram", [HW, 64], f32)

    with tile.TileContext(nc, num_cores=N_CORES) as tc, ExitStack() as top:
        wpool = top.enter_context(tc.tile_pool(name="weights", bufs=1))
        # SP queue: blobA, consts, gix (small, early)
        blobA = wpool.tile([128, CA], fp8)
        nc.sync.dma_start(out=blobA, in_=d_blobA[:, :])
        if with_b2:
            b2r_t = wpool.tile([1, 64], fp8)
            nc.sync.dma_start(out=b2r_t, in_=d_b2r[:, :])
            svr_t = wpool.tile([1, PADQ, 128], fp8)
            nc.sync.dma_start(out=svr_t, in_=d_svr[:, :, :])

        nc.gpsimd.load_library(mlp_lib)

        # Pool DMA queue: stk bands, then blobB chunks (dgt first)
        stk_t = wpool.tile([99, 2 * PIMG], fp8)
        CHK = 2 * 16 * PADW
        for bb in range(8):
            nc.sync.dma_start(out=stk_t[:, bb * CHK:(bb + 1) * CHK],
                              in_=d_stk[:, bb * CHK:(bb + 1) * CHK])
        consts = wpool.tile([128, CC], f32)
        nc.sync.dma_start(out=consts, in_=d_consts[:, :])
        gix_t = wpool.tile([128, 2 * (NV // 16)], i16)
        nc.sync.dma_start(out=gix_t, in_=d_gix[:, :])
        blobB = wpool.tile([128, CB], fp8)
        nc.gpsimd.dma_start(out=blobB[:, CB_DGT:CB_RES],
                            in_=d_blobB[:, CB_DGT:CB_RES])
        for i in range(NRES):
            c0 = CB_RES + i * 1280
            nc.gpsimd.dma_start(out=blobB[:, c0:c0 + 1280],
                                in_=d_blobB[:, c0:c0 + 1280])
        nc.gpsimd.dma_start(out=blobB[:, CB_FUS:CB],
                            in_=d_blobB[:, CB_FUS:CB])

        aps = blobA.ap[0][0]
        bps = blobB.ap[0][0]
        cps = consts.ap[0][0]

        def ap_of(blob, pstride, parts, col0, dims):
            return bass.AP(tensor=blob.tensor, offset=blob.offset + col0,
                           ap=[[pstride, parts]] + dims)

        def w1p_ap(p, m):
            return ap_of(blobA, aps, 99, CA_W1P + p * 512 + m * 128,
                         [[256, 2], [1, 128]])

        def w2t_ap():
            return ap_of(blobA, aps, 128, CA_W2T, [[64, 2], [1, 64]])

        def headw_ap(p):
            return ap_of(blobA, aps, 66, CA_HW + p * 256, [[128, 2], [1, 128]])

        def pw3_ap():
            return ap_of(blobA, aps, 64, CA_PW3, [[1, 2]])

        def dgt_ap(q, gi):
            return ap_of(blobB, bps, 128, CB_DGT + q * 512 + gi * 256,
                         [[128, 2], [1, 128]])

        def resw_ap(i, p, m):
            return ap_of(blobB, bps, 128, CB_RES + i * 1280 + p * 256,
                         [[128, 2], [1, 128]])

        def fusw_ap(k, m):
            return ap_of(blobB, bps, 128, CB_FUS + k * 512 + m * 128,
                         [[256, 2], [1, 128]])

        def pw1_ap(p, m):
            return ap_of(blobB, bps, 128, CB_PW1 + p * 512 + m * 128,
                         [[256, 2], [1, 128]])

        def pw2_ap():
            return ap_of(blobB, bps, 128, CB_PW2, [[64, 2], [1, 64]])

        def const_col(c0, parts=128, n=1):
            return ap_of(consts, cps, parts, c0, [[1, n]])

        base_ap = ap_of(consts, cps, 128, CC_BASE, [[2, PADQ], [1, 2]])

        # --- engine helpers ---
        rr_state = [0]

        def rr_relu(out_ap, in_ap, scale, bias_ap, force=None):
            """out = relu(scale*x + bias). scale may be const or AP.
            If zb (bias==0), can run on act or dve; else activation only.
            (GPSIMD cannot access PSUM, so pool never does these.)"""
            if not zb or force == 'act':
                nc.scalar.activation(out_ap, in_ap, AF.Relu,
                                     bias=(bias_ap if (bias_ap is not None and
                                                       not zb) else 0.0),
                                     scale=scale)
                return
            e = force if force is not None else ('act', 'dve')[rr_state[0] % 2]
            if force is None:
                rr_state[0] += 1
            if e == 'act':
                nc.scalar.activation(out_ap, in_ap, AF.Relu, bias=0.0,
                                     scale=scale)
            else:
                nc.vector.tensor_scalar(out_ap, in_ap, scale, 0.0,
                                        op0=ALU.mult, op1=ALU.max)

        def rr_copy(out_ap, in_ap, scale=None, force=None, pool_ok=False):
            engs = ('act', 'dve', 'pool') if pool_ok else ('act', 'dve')
            e = force if force is not None else engs[rr_state[0] % len(engs)]
            if force is None:
                rr_state[0] += 1
            if scale is None:
                if e == 'act':
                    nc.scalar.activation(out_ap, in_ap, AF.Copy, bias=0.0)
                elif e == 'dve':
                    nc.vector.tensor_copy(out_ap, in_ap)
                else:
                    nc.gpsimd.tensor_copy(out_ap, in_ap)
            else:
                if e == 'act':
                    nc.scalar.activation(out_ap, in_ap, AF.Copy, bias=0.0,
                                         scale=scale)
                elif e == 'dve':
                    nc.vector.tensor_scalar(out_ap, in_ap, scale, None,
                                            op0=ALU.mult)
                else:
                    nc.gpsimd.tensor_scalar(out_ap, in_ap, scale, None,
                                            op0=ALU.mult)

        # ------------ conv1 (3x3, 66->256) + conv2 (1x1, 256->64) ------------
        with tc.tile_pool(name="conv", bufs=1) as cpool, \
             tc.tile_pool(name="psumA", bufs=2, space="PSUM") as ppA, \
             tc.tile_pool(name="psumB", bufs=2, space="PSUM") as ppB, \
             tc.tile_pool(name="stage", bufs=3) as spool:
            r1 = cpool.tile([128, 2, HW], fp8)
            ps2 = {}

            def emit_conv2(g):
                h = g % 2
                if h == 0:
                    ps2[0] = ppB.tile([128, 8, 64], f32, tag="c2", name="c2")
                for cc in range(4):
                    pos0 = g * 512 + cc * 128
                    lhsT = bass.AP(tensor=r1.tensor, offset=r1.offset + pos0,
                                   ap=[r1.ap[0], [HW, 2], [1, 128]])
                    nc.tensor.matmul(ps2[0][:, h * 4 + cc, :], lhsT, w2t_ap(),
                                     start=True, stop=True, perf_mode=DR)
                if h == 1:
                    stg = spool.tile([128, 8, 64], f32, tag="stage", name="stg")
                    rr_copy(stg, ps2[0], 1.0 / (A_W * S_R1))
                    dst = bass.AP(tensor=feat_dram,
                                  offset=((g - 1) * 512) * 64,
                                  ap=[[64, 128], [8192, 8], [1, 64]])
                    nc.sync.dma_start(out=dst, in_=stg)

            for g in range(32):              # y-groups of 4 rows
                for m in range(2):
                    pg = ppA.tile([128, 4, 128], f32, tag=f"c1_{m}",
                                  name=f"c1_{m}")
                    for yy in range(4):
                        y = 4 * g + yy
                        ktaps = ((2 * y * PADW, 2), (2 * y * PADW + 1, 3),
                                 (2 * (y * PADW + 1) + 1, 2))
                        for p, (off, stride) in enumerate(ktaps):
                            rhs = bass.AP(tensor=stk_t.tensor,
                                          offset=stk_t.offset + off,
                                          ap=[stk_t.ap[0], [stride, 2],
                                              [2, 128]])
                            nc.tensor.matmul(pg[:, yy, :], w1p_ap(p, m),
                                             rhs, start=(p == 0), stop=(p == 2),
                                             perf_mode=DR)
                    rr_relu(r1[:, m, g * 512:(g + 1) * 512],
                            pg.rearrange("p a b -> p (a b)"), S_R1 / A_W,
                            const_col(CC_PB0 + m))
                if g >= 2:
                    emit_conv2(g - 2)        # skewed to avoid PE queue stall
            emit_conv2(30)
            emit_conv2(31)

        # ------------ gather + combine-transpose into ipad ------------
        with tc.tile_pool(name="snake", bufs=1) as sn:
            ipad = sn.tile([128, PADQ, PADV], fp8, tag="ipad", name="ipad")
            zall = sn.tile([128, 8, PADQ, PADV], fp8, tag="zall", name="zall")
            rsc = sn.tile([128, PADQ, 128], fp8, tag="rsc", name="rsc")

            with tc.tile_pool(name="gat", bufs=1) as gp, \
                 tc.tile_pool(name="psumG", bufs=4, space="PSUM") as ppG, \
                 tc.tile_pool(name="psumS", bufs=4, space="PSUM") as ppS:
                gta = gp.tile([128, PADQ, 128], f32, tag="gta", name="gta")
                gtb = gp.tile([128, PADQ, 128], f32, tag="gtb", name="gtb")
                g8a = gp.tile([128, PADQ, 128], fp8, tag="g8a", name="g8a")
                g8b = gp.tile([128, PADQ, 128], fp8, tag="g8b", name="g8b")
                gsrc = bass.AP(tensor=feat_dram, offset=0,
                               ap=[[64, HW - 1], [1, 128]])
                HQ = PADQ // 2
                HV = NV // 2
                for hh in range(2):
                    qh = slice(hh * HQ, (hh + 1) * HQ)
                    nc.gpsimd.dma_gather(
                        gta[:, qh, :], gsrc,
                        gix_t[:, hh * HV // 16:(hh + 1) * HV // 16],
                        HV, HV, 128, elem_step=64, single_packet=False)
                    nc.gpsimd.dma_gather(
                        gtb[:, qh, :], gsrc,
                        gix_t[:, (2 + hh) * HV // 16:(3 + hh) * HV // 16],
                        HV, HV, 128, elem_step=64, single_packet=False)
                nc.sync.dma_start(out=ipad[64:66, :, :], in_=d_cpv[:, :, :])

                def emit_gather_block(qb):
                    qsl = slice(4 * qb, 4 * qb + 4)
                    rr_copy(g8a[:, qsl, :], gta[:, qsl, :], scale=S_FEAT,
                            pool_ok=True)
                    rr_copy(g8b[:, qsl, :], gtb[:, qsl, :], scale=S_FEAT,
                            pool_ok=True)
                    pg = ppG.tile([64, 4, 128], f32, tag="dg", name="dg")
                    for qq in range(4):
                        q = 4 * qb + qq
                        last = 2 if with_b2 else 1
                        for gi, gt in enumerate((g8a, g8b)):
                            lhsT = bass.AP(
                                tensor=gt.tensor,
                                offset=gt.offset + q * 128,
                                ap=[gt.ap[0], [64, 2], [1, 64]])
                            nc.tensor.matmul(pg[:, qq, :], lhsT,
                                             dgt_ap(q, gi),
                                             start=(gi == 0), stop=(gi == last),
                                             perf_mode=DR)
                        if with_b2:
                            nc.tensor.matmul(pg[:, qq, :], b2r_t[:, :],
                                             svr_t[:, q, :], start=False,
                                             stop=True)
                    rr_copy(ipad[0:64, qsl, 16:144], pg)
                    weng = nc.vector if qb % 2 == 0 else nc.gpsimd
                    weng.tensor_copy(ipad[0:66, qsl, 0:16],
                                     ipad[0:66, qsl, 128:144])
                    weng.tensor_copy(ipad[0:66, qsl, 144:160],
                                     ipad[0:66, qsl, 16:32])

                def conv_layer_qb(zo, rhs_base_fn, rhs_tensor, rhs_ap0,
                                  lhsT_fn, dil, src_zi, qb):
                    ps = ppS.tile([128, 4, 128], f32, tag="psS", name="psS")
                    for qq in range(4):
                        q = 4 * qb + qq
                        for p in range(5):
                            if p < 4:
                                off = rhs_base_fn(q) + 16 + (2 * p - 4) * dil
                                stride = dil
                            else:
                                off = rhs_base_fn(q) + 16 + 4 * dil
                                stride = 0
                            rhs = bass.AP(tensor=rhs_tensor, offset=off,
                                          ap=[rhs_ap0, [stride, 2], [1, 128]])
                            nc.tensor.matmul(ps[:, qq, :], lhsT_fn(p), rhs,
                                             start=(p == 0), stop=(p == 4),
                                             perf_mode=DR)
                    qsl = slice(4 * qb, 4 * qb + 4)
                    scale_ap = const_col(CC_LSB + 2 * zo)
                    bias_ap = const_col(CC_LSB + 2 * zo + 1)
                    if src_zi is None:
                        rr_relu(zall[:, 0, qsl, 16:144], ps, scale_ap,
                                bias_ap, force=('act' if qb != 4 else 'dve'))
                    else:
                        rr_relu(rsc[:, qsl, :], ps, scale_ap, bias_ap,
                                force=('act' if qb != 4 else 'dve'))
                        addeng = nc.gpsimd if qb in (1, 2, 3, 4) else nc.vector
                        addeng.tensor_tensor(
                            zall[:, zo, qsl, 16:144],
                            zall[:, src_zi, qsl, 16:144],
                            rsc[:, qsl, :], ALU.add)
                    weng = nc.vector if qb % 2 == 0 else nc.gpsimd
                    weng.tensor_copy(zall[:, zo, qsl, 0:16],
                                     zall[:, zo, qsl, 128:144])
                    weng.tensor_copy(zall[:, zo, qsl, 144:160],
                                     zall[:, zo, qsl, 16:32])

                ip66 = ipad[0:66, :, :]

                def head_qb(qb):
                    conv_layer_qb(0, lambda q: ip66.offset + q * PADV,
                                  ip66.tensor, ip66.ap[0], headw_ap, 1, None,
                                  qb)

                # interleave: gather blocks then head conv per poly-half
                for qb in range(NQB // 2):
                    emit_gather_block(qb)
                for qb in range(NQB // 2):
                    head_qb(qb)
                for qb in range(NQB // 2, NQB):
                    emit_gather_block(qb)
                for qb in range(NQB // 2, NQB):
                    head_qb(qb)

                for i in range(NRES):
                    zi_off = zall.offset + i * PADQ * PADV
                    for qb in range(NQB):
                        conv_layer_qb(i + 1,
                                      lambda q, zi_off=zi_off: zi_off + q * PADV,
                                      zall.tensor, zall.ap[0],
                                      lambda p, i=i: resw_ap(i, p, 0),
                                      DIL[i], i, qb)

                # fusion 1x1 (1024->256) + per-poly max over V
                gmax = sn.tile([128, 2, PADQ], f32, tag="gmax", name="gmax")
                gb = sn.tile([128, 2, PADQ], fp8, tag="gb", name="gb")
                for m in range(2):
                    for qb in range(NQB):
                        ps = ppS.tile([128, 4, 128], f32, tag="psS", name="psS")
                        for qq in range(4):
                            q = 4 * qb + qq
                            for k in range(4):
                                off = zall.offset + (2 * k * PADQ + q) * PADV + 16
                                rhs = bass.AP(tensor=zall.tensor, offset=off,
                                              ap=[zall.ap[0],
                                                  [PADQ * PADV, 2], [1, 128]])
                                nc.tensor.matmul(ps[:, qq, :],
                                                 fusw_ap(k, m), rhs,
                                                 start=(k == 0), stop=(k == 3),
                                                 perf_mode=DR)
                        nc.vector.tensor_reduce(gmax[:, m, 4 * qb:4 * qb + 4],
                                                ps, axis=mybir.AxisListType.X,
                                                op=ALU.max)
                    nc.vector.tensor_scalar(gb[:, m, :], gmax[:, m, :],
                                            S_GB / (A_W * S_Z),
                                            const_col(CC_FUSC + m),
                                            op0=ALU.mult, op1=ALU.add)

                # pred1: 1280 -> 256 relu
                h1 = sn.tile([128, 2, PADQ, 128], fp8, tag="h1", name="h1")
                for m in range(2):
                    for qb in range(NQB):
                        ps = ppS.tile([128, 4, 128], f32, tag="psS", name="psS")
                        for qq in range(4):
                            q = 4 * qb + qq
                            for k in range(4):
                                off = zall.offset + (2 * k * PADQ + q) * PADV + 16
                                rhs = bass.AP(tensor=zall.tensor, offset=off,
                                              ap=[zall.ap[0],
                                                  [PADQ * PADV, 2], [1, 128]])
                                nc.tensor.matmul(ps[:, qq, :],
                                                 pw1_ap(k + 1, m), rhs,
                                                 start=(k == 0), stop=False,
                                                 perf_mode=DR)
                            rhs0 = bass.AP(tensor=gb.tensor,
                                           offset=gb.offset + q,
                                           ap=[gb.ap[0], [PADQ, 2], [0, 128]])
                            nc.tensor.matmul(ps[:, qq, :], pw1_ap(0, m),
                                             rhs0, start=False, stop=True,
                                             perf_mode=DR)
                        rr_relu(h1[:, m, 4 * qb:4 * qb + 4, :], ps,
                                S_H1 / (A_W * S_Z), const_col(CC_PB1 + m))

            # pred2 + pred3
            with tc.tile_pool(name="psumT", bufs=3, space="PSUM") as ppT:
                h2 = sn.tile([64, PADQ, 128], fp8, tag="h2", name="h2")
                for qb in range(NQB):
                    ps = ppT.tile([64, 4, 128], f32, tag="psT", name="psT")
                    for qq in range(4):
                        q = 4 * qb + qq
                        rhs = bass.AP(tensor=h1.tensor,
                                      offset=h1.offset + q * 128,
                                      ap=[h1.ap[0], [PADQ * 128, 2], [1, 128]])
                        nc.tensor.matmul(ps[:, qq, :], pw2_ap(), rhs,
                                         start=True, stop=True, perf_mode=DR)
                    rr_relu(h2[:, 4 * qb:4 * qb + 4, :], ps,
                            S_H2 / (A_W * S_H1), const_col(CC_PB2, parts=64))

                ps3 = ppT.tile([128, PADQ, 2], f32, tag="psT3", name="psT3",
                               bufs=1)
                for q in range(PADQ):
                    nc.tensor.matmul(ps3[:, q, :], h2[:, q, :], pw3_ap(),
                                     start=True, stop=True)
                o_f = sn.tile([128, PADQ, 2], f32, tag="o_f", name="o_f")
                nc.vector.tensor_scalar(o_f, ps3, 1.0 / (A_W * S_H2), None,
                                        op0=ALU.mult)
                o_t = sn.tile([128, PADQ, 2], f32, tag="o_t", name="o_t")
                nc.vector.tensor_tensor(o_t, o_f, base_ap, ALU.add)
                nc.sync.dma_start(out=d_out[:, :, :], in_=o_t)

    nc.compile()
    return nc


_NC_CACHE = {}


def _get_nc_key(P, with_b2, zb):
    key = (P, with_b2, zb)
    if key not in _NC_CACHE:
        _NC_CACHE[key] = build_nc(P, with_b2, zb)
    return _NC_CACHE[key]


def _get_nc(P):
    """test.py compatibility: default flags for the standard input set."""
    return _get_nc_key(P, False, True)


def _host_prep(inputs, P, counts, order, offs):
    """Build per-core in_maps. Returns (in_maps, with_b2, zb)."""
    PADQ = -(-P // 4) * 4
    NV = PADQ * 128
    cnn = np.asarray(inputs["cnn_feature"], np.float32)
    ipoly = np.asarray(inputs["i_it_poly"], np.float32)
    cpoly = np.asarray(inputs["c_it_poly"], np.float32)
    w1 = np.asarray(inputs["proj_w1"], np.float32)
    pb0 = np.asarray(inputs["proj_b1"], np.float32)
    w2 = np.asarray(inputs["proj_w2"], np.float32)[:, :, 0, 0]  # [64, 256]
    b2 = np.asarray(inputs["proj_b2"], np.float32)
    head_w = np.asarray(inputs["head_w"], np.float32)   # [128, 66, 9]
    head_b = np.asarray(inputs["head_b"], np.float32)
    head_g = np.asarray(inputs["head_g"], np.float32)
    head_bt = np.asarray(inputs["head_bt"], np.float32)
    res_w = np.asarray(inputs["res_w"], np.float32)     # [7, 128, 128, 9]
    res_b = np.asarray(inputs["res_b"], np.float32)
    res_g = np.asarray(inputs["res_g"], np.float32)
    res_bt = np.asarray(inputs["res_bt"], np.float32)
    fus_w = np.asarray(inputs["fus_w"], np.float32)     # [256, 1024]
    fus_b = np.asarray(inputs["fus_b"], np.float32)
    pw1 = np.asarray(inputs["pw1"], np.float32)         # [256, 1280]
    pb1 = np.asarray(inputs["pb1"], np.float32)
    pw2 = np.asarray(inputs["pw2"], np.float32)         # [64, 256]
    pb2 = np.asarray(inputs["pb2"], np.float32)
    pw3 = np.asarray(inputs["pw3"], np.float32)         # [2, 64]
    pb3 = np.asarray(inputs["pb3"], np.float32)

    assert (head_g > 0).all() and (res_g > 0).all(), "bn fold requires g>0"
    with_b2 = bool(np.any(b2 != 0))

    # w1p [99, 3pairs, 2kt, 2m, 128]
    w1p = np.zeros((99, 3, 2, 2, 128), np.float32)
    pair_src = [((0, 0), (0, 1)), ((1, 0), (0, 2)), ((1, 1), (1, 2))]
    for p, pr in enumerate(pair_src):
        for kt, (blk, kw) in enumerate(pr):
            rm = ROWMAP_A if blk == 0 else ROWMAP_B
            for r in range(99):
                ch, kh = rm[r]
                for m in range(2):
                    w1p[r, p, kt, m, :] = A_W * w1[m * 128:(m + 1) * 128,
                                                   ch, kh, kw]
    pb0s = (S_R1 * pb0).reshape(2, 128).T.copy()

    w2t = np.zeros((128, 2, 64), np.float32)
    for kt in range(2):
        w2t[:, kt, :] = A_W * w2[:, kt * 128:(kt + 1) * 128].T

    # ---- grid-sample host math ----
    ix = ipoly[..., 0] - np.float32(0.5)
    iy = ipoly[..., 1] - np.float32(0.5)
    x0 = np.floor(ix); y0 = np.floor(iy)
    wx = (ix - x0).astype(np.float32); wy = (iy - y0).astype(np.float32)
    x0i = x0.astype(np.int64); y0i = y0.astype(np.int64)

    swap_x = x0i < 0
    vx0 = (x0i >= 0) & (x0i < W)
    vx1 = (x0i + 1 >= 0) & (x0i + 1 < W)

    def slot_weights(yi):
        vy = (yi >= 0) & (yi < H)
        w_s0 = (1 - wx) * vx0 * vy
        w_s1 = wx * vx1 * vy
        w_s0 = np.where(swap_x, wx * vx1 * vy, w_s0)
        w_s1 = np.where(swap_x, 0.0, w_s1)
        return w_s0.astype(np.float32), w_s1.astype(np.float32)

    x0c = np.clip(x0i, 0, W - 2)
    y0c = np.clip(y0i, 0, H - 1)
    y1c = np.clip(y0i + 1, 0, H - 1)
    idxA = (y0c * W + x0c).astype(np.int64)          # [NP, V]
    idxB = (y1c * W + x0c).astype(np.int64)
    wA0, wA1 = slot_weights(y0i)
    wB0, wB1 = slot_weights(y0i + 1)
    wA0 *= (1 - wy); wA1 *= (1 - wy)
    wB0 *= wy; wB1 *= wy
    s_v = wA0 + wA1 + wB0 + wB1

    # ---- snake weights (bn + scale folds) ----
    headw = np.zeros((66, 5, 2, 128), np.float32)
    hw9 = head_w.transpose(1, 2, 0)                  # [66, 9, 128]
    for t in range(9):
        p, kt = t // 2, t % 2
        headw[0:64, p, kt, :] = A_W * hw9[0:64, t, :]
        headw[64:66, p, kt, :] = A_W * S_FEAT * hw9[64:66, t, :]
    lsb = np.zeros((128, 8, 2), np.float32)
    lsb[:, 0, 0] = head_g / A_W
    lsb[:, 0, 1] = S_Z * head_g * head_b
    C = np.zeros((8, 128), np.float32)               # C_i = sum_{j<=i} bt_j
    C[0] = head_bt
    for i in range(NRES):
        C[i + 1] = C[i] + res_bt[i]

    resw = np.zeros((128, 7, 5, 2, 128), np.float32)
    for i in range(NRES):
        rw = res_w[i].transpose(1, 2, 0)             # [128 in, 9, 128 out]
        for t in range(9):
            p, kt = t // 2, t % 2
            resw[:, i, p, kt, :] = A_W * rw[:, t, :]
        bprime = res_b[i] + res_w[i].sum(axis=2) @ C[i]
        lsb[:, i + 1, 0] = res_g[i] / A_W
        lsb[:, i + 1, 1] = S_Z * res_g[i] * bprime

    fw8 = fus_w.reshape(256, 8, 128)
    fusw = np.zeros((128, 4, 2, 2, 128), np.float32)
    for k in range(4):
        for kt in range(2):
            for m in range(2):
                fusw[:, k, kt, m, :] = A_W * fw8[m * 128:(m + 1) * 128,
                                                 2 * k + kt].T
    fusconst = fus_b + np.einsum('ojc,jc->o', fw8, C)
    fusc = (S_GB * fusconst).reshape(2, 128).T.copy()

    pw1r = pw1.reshape(256, 10, 128)
    pw1p = np.zeros((128, 5, 2, 2, 128), np.float32)
    for m in range(2):
        for kt in range(2):
            pw1p[:, 0, kt, m, :] = (A_W * S_Z / S_GB) * \
                pw1r[m * 128:(m + 1) * 128, kt].T
        for k in range(4):
            for kt in range(2):
                pw1p[:, k + 1, kt, m, :] = A_W * \
                    pw1r[m * 128:(m + 1) * 128, 2 + 2 * k + kt].T
    pb1prime = pb1 + np.einsum('ojc,jc->o', pw1r[:, 2:], C)
    pb1s = (S_H1 * pb1prime).reshape(2, 128).T.copy()

    pw2t = np.zeros((128, 2, 64), np.float32)
    for kt in range(2):
        pw2t[:, kt, :] = A_W * pw2[:, kt * 128:(kt + 1) * 128].T
    pb2s = (S_H2 * pb2).reshape(64, 1)
    pw3t = A_W * pw3.T                                # [64, 2]

    zb = (not np.any(pb0)) and (not np.any(lsb[:, :, 1])) \
        and (not np.any(pb1s)) and (not np.any(pb2s))

    # ---- pack blobs ----
    CA = 2946
    blobA = np.zeros((128, CA), F8)
    blobA[0:99, 0:1536] = _f8(w1p).reshape(99, -1)
    blobA[:, 1536:1664] = _f8(w2t).reshape(128, -1)
    blobA[0:66, 1664:2944] = _f8(headw).reshape(66, -1)
    blobA[0:64, 2944:2946] = _f8(pw3t)

    CB_RES = PADQ * 512
    CB_FUS = CB_RES + 8960
    CB_PW1 = CB_FUS + 2048
    CB_PW2 = CB_PW1 + 2560
    CB = CB_PW2 + 128
    blobB_shared = np.zeros((128, CB - CB_RES), F8)
    blobB_shared[:, 0:8960] = _f8(resw).reshape(128, -1)
    blobB_shared[:, 8960:8960 + 2048] = _f8(fusw).reshape(128, -1)
    blobB_shared[:, 11008:11008 + 2560] = _f8(pw1p).reshape(128, -1)
    pw2pad = np.zeros((128, 2, 64), np.float32)
    pw2pad[:, :, :] = pw2t[:, :, 0:64]
    blobB_shared[:, 13568:13696] = _f8(pw2pad).reshape(128, -1)

    CC = 24 + PADQ * 2
    consts_shared = np.zeros((128, CC), np.float32)
    consts_shared[:, 0:2] = pb0s
    consts_shared[:, 2:18] = lsb.reshape(128, -1)
    consts_shared[:, 18:20] = fusc
    consts_shared[:, 20:22] = pb1s
    consts_shared[0:64, 22:23] = pb2s

    shared = {"blobA": blobA}
    if with_b2:
        shared["b2r"] = _f8(S_FEAT * b2.reshape(1, 64))

    in_maps = []
    for c in range(N_CORES):
        img = cnn[c]
        img_pad = np.zeros((C_IN, PADW, PADW), np.float32)
        img_pad[:, 1:129, 1:129] = img
        flatf = _f8(img_pad.reshape(C_IN, PIMG)).astype(np.float32)
        stk = np.zeros((99, PIMG, 2), np.float32)
        for r in range(99):
            ch, kh = ROWMAP_A[r]
            ln = PIMG - kh * PADW
            stk[r, :ln, 0] = flatf[ch, kh * PADW:]
            ch, kh = ROWMAP_B[r]
            ln = PIMG - kh * PADW
            stk[r, :ln, 1] = flatf[ch, kh * PADW:]
        stk = stk.reshape(99, 2 * PIMG)

        own = order[offs[c]:offs[c + 1]]
        nown = len(own)
        gixa = np.zeros(NV, np.int64)
        gixb = np.zeros(NV, np.int64)
        dgt = np.zeros((128, PADQ, 4, 128), np.float32)
        ar = np.arange(128)
        for qi, poly in enumerate(own):
            gixa[qi * 128:(qi + 1) * 128] = idxA[poly]
            gixb[qi * 128:(qi + 1) * 128] = idxB[poly]
            dgt[ar, qi, 0, ar] = wA0[poly]
            dgt[ar, qi, 1, ar] = wA1[poly]
            dgt[ar, qi, 2, ar] = wB0[poly]
            dgt[ar, qi, 3, ar] = wB1[poly]

        cpv = np.zeros((2, PADQ, 160), np.float32)
        if nown:
            cc = (cpoly[own] * RO).transpose(2, 0, 1)     # [2, nown, 128]
            cpv[:, :nown, 16:144] = cc
            cpv[:, :nown, 0:16] = cc[:, :, 112:128]
            cpv[:, :nown, 144:160] = cc[:, :, 0:16]

        base = np.zeros((128, PADQ, 2), np.float32)
        if nown:
            base[:, :nown, :] = (ipoly[own] * RO + pb3[None, None, :]) \
                .transpose(1, 0, 2)

        blobB = np.zeros((128, CB), F8)
        blobB[:, 0:CB_RES] = _f8(dgt).reshape(128, -1)
        blobB[:, CB_RES:] = blobB_shared
        consts = consts_shared.copy()
        consts[:, 24:] = base.reshape(128, -1)
        gix = np.concatenate([pack16(gixa, NV // 16),
                              pack16(gixb, NV // 16)], axis=1)
        m = {
            "stk": _f8(stk), "blobB": blobB, "consts": consts,
            "gix": gix, "cpv": _f8(cpv),
        }
        if with_b2:
            svr = np.zeros((1, PADQ, 128), np.float32)
            svr[0, :nown, :] = s_v[own]
            m["svr"] = _f8(svr)
        m.update(shared)
        in_maps.append(m)
    return in_maps, with_b2, zb


def kernel(**inputs):
    ind = np.asarray(inputs["ind"]).astype(np.int64)
    counts = np.bincount(ind, minlength=N_CORES)
    P = int(counts.max())
    order = np.argsort(ind, kind="stable")
    offs = np.concatenate([[0], np.cumsum(counts)])

    in_maps, with_b2, zb = _host_prep(inputs, P, counts, order, offs)
    nc = _get_nc_key(P, with_b2, zb)
    res = None
    last_err = None
    for _attempt in range(3):
        try:
            res = run_bass_kernel_spmd(nc, in_maps, list(range(N_CORES)))
            break
        except Exception as e:  # rare transient device error; retry
            last_err = e
    if res is None:
        raise last_err

    out = np.zeros((NP, V, 2), np.float32)
    for c in range(N_CORES):
        oc = res.results[c]["out"]  # [128v, PADQ, 2]
        own = order[offs[c]:offs[c + 1]]
        for q, opoly in enumerate(own):
            out[opoly] = oc[:, q, :]
    return out


# revision 17
# speedup vs baseline: 1.3110x; 1.0591x over previous
"""Trainium2 Bass kernel for nn_Evolution_26697516712465 (deep-snake GNN).

Self-contained: takes FULL inputs, shards batch across 8 NeuronCores internally
(one image per core; each core runs the snake for the polys of its own image),
returns FULL output [128, 128, 2] fp32.

fp8e4 (e4m3) DoubleRow matmuls throughout (2 contraction rows/cycle), weights
pre-scaled by 64 into fp8 normal range, activations carried at power-of-2
scales; bilinear grid-sample folded into PE "diagonal" matmuls on gathered
corner row-pairs; eval-mode bn folded into weights/biases host-side.
"""
import numpy as np
import ml_dtypes
from contextlib import ExitStack

import concourse.bass as bass
import concourse.bacc as bacc
import concourse.mybir as mybir
import concourse.tile as tile
from concourse.library_config import mlp as mlp_lib
from concourse.bass_utils import run_bass_kernel_spmd

N_CORES = 8
B, C_IN, H, W = 8, 66, 128, 128
NP, V = 128, 128
RO = 4.0
DIL = (1, 1, 1, 2, 2, 4, 4)
NRES = 7
HW = H * W          # 16384
PADW = W + 2        # 130
PIMG = PADW * PADW  # 16900
PADV = 160          # 16 + 128 + 16 circular pad

f32 = mybir.dt.float32
f32r = mybir.dt.float32r
fp8 = mybir.dt.float8e4
i16 = mybir.dt.int16
AF = mybir.ActivationFunctionType
ALU = mybir.AluOpType
DR = mybir.MatmulPerfMode.DoubleRow

F8 = ml_dtypes.float8_e4m3

# activation/weight scales (powers of 2)
A_W = 64.0          # weight scale
S_R1 = 8.0          # conv1 relu out
S_FEAT = 32.0       # ipad feat rows (folded into diag weights)
S_Z = 32.0          # snake states
S_GB = 128.0        # fusion global feature
S_H1 = 128.0
S_H2 = 512.0

# conv1 stack row maps: blkA = 66ch kh0 + 33ch kh1; blkB = 33ch kh1 + 66ch kh2
ROWMAP_A = [(r, 0) if r < 66 else (r - 66, 1) for r in range(99)]
ROWMAP_B = [(r + 33, 1) if r < 33 else (r - 33, 2) for r in range(99)]


def _f8(x):
    return np.clip(np.asarray(x, np.float32), -240.0, 240.0).astype(F8)


def pack16(idx_flat, cols):
    tab = np.zeros((16, cols), np.int16)
    n = len(idx_flat)
    tab[np.arange(n) % 16, np.arange(n) // 16] = idx_flat.astype(np.int16)
    return np.tile(tab, (8, 1))


def build_nc(P, with_b2, zb):
    """Build the SPMD Bass program. P = max polys per image; zb = all relu
    biases are zero (allows relu on DVE/Pool engines)."""
    nc = bacc.Bacc("TRN2", target_bir_lowering=False, debug=False)
    PADQ = -(-P // 4) * 4
    NQB = PADQ // 4
    NV = PADQ * 128

    # ---------------- inputs ----------------
    # blobA: w1p | w2t | headw | pw3 (fp8, conv+early weights)
    CA_W1P, CA_W2T, CA_HW, CA_PW3 = 0, 1536, 1664, 2944
    CA = 2946
    # blobB: dgt | resw | fusw | pw1 | pw2 (fp8, late weights)
    CB_DGT = 0
    CB_RES = PADQ * 512
    CB_FUS = CB_RES + 8960
    CB_PW1 = CB_FUS + 2048
    CB_PW2 = CB_PW1 + 2560
    CB = CB_PW2 + 128
    # consts (f32): pb0 | lsb | fusc | pb1 | pb2 | base
    CC_PB0, CC_LSB, CC_FUSC, CC_PB1, CC_PB2, CC_BASE = 0, 2, 18, 20, 22, 24
    CC = 24 + PADQ * 2
    d_stk = nc.declare_dram_parameter("stk", [99, 2 * PIMG], fp8, isOutput=False)
    d_blobA = nc.declare_dram_parameter("blobA", [128, CA], fp8, isOutput=False)
    d_blobB = nc.declare_dram_parameter("blobB", [128, CB], fp8, isOutput=False)
    d_consts = nc.declare_dram_parameter("consts", [128, CC], f32, isOutput=False)
    d_gix = nc.declare_dram_parameter("gix", [128, 2 * (NV // 16)], i16, isOutput=False)
    d_cpv = nc.declare_dram_parameter("cpv", [2, PADQ, 160], fp8, isOutput=False)
    if with_b2:
        d_b2r = nc.declare_dram_parameter("b2r", [1, 64], fp8, isOutput=False)
        d_svr = nc.declare_dram_parameter("svr", [1, PADQ, 128], fp8, isOutput=False)
    d_out = nc.declare_dram_parameter("out", [128, PADQ, 2], f32, isOutput=True)

    feat_dram = nc.dram_tensor("feat_dram", [HW, 64], f32)

    with tile.TileContext(nc, num_cores=N_CORES) as tc, ExitStack() as top:
        wpool = top.enter_context(tc.tile_pool(name="weights", bufs=1))
        # SP queue: blobA, consts, gix (small, early)
        blobA = wpool.tile([128, CA], fp8)
        nc.sync.dma_start(out=blobA, in_=d_blobA[:, :])
        if with_b2:
            b2r_t = wpool.tile([1, 64], fp8)
            nc.sync.dma_start(out=b2r_t, in_=d_b2r[:, :])
            svr_t = wpool.tile([1, PADQ, 128], fp8)
            nc.sync.dma_start(out=svr_t, in_=d_svr[:, :, :])

        nc.gpsimd.load_library(mlp_lib)

        # Pool DMA queue: stk bands, then blobB chunks (dgt first)
        stk_t = wpool.tile([99, 2 * PIMG], fp8)
        CHK = 2 * 16 * PADW
        for bb in range(8):
            nc.sync.dma_start(out=stk_t[:, bb * CHK:(bb + 1) * CHK],
                              in_=d_stk[:, bb * CHK:(bb + 1) * CHK])
        consts = wpool.tile([128, CC], f32)
        nc.sync.dma_start(out=consts, in_=d_consts[:, :])
        gix_t = wpool.tile([128, 2 * (NV // 16)], i16)
        nc.sync.dma_start(out=gix_t, in_=d_gix[:, :])
        blobB = wpool.tile([128, CB], fp8)
        nc.gpsimd.dma_start(out=blobB[:, CB_DGT:CB_RES],
                            in_=d_blobB[:, CB_DGT:CB_RES])
        for i in range(NRES):
            c0 = CB_RES + i * 1280
            nc.gpsimd.dma_start(out=blobB[:, c0:c0 + 1280],
                                in_=d_blobB[:, c0:c0 + 1280])
        nc.gpsimd.dma_start(out=blobB[:, CB_FUS:CB],
                            in_=d_blobB[:, CB_FUS:CB])

        aps = blobA.ap[0][0]
        bps = blobB.ap[0][0]
        cps = consts.ap[0][0]

        def ap_of(blob, pstride, parts, col0, dims):
            return bass.AP(tensor=blob.tensor, offset=blob.offset + col0,
                           ap=[[pstride, parts]] + dims)

        def w1p_ap(p, m):
            return ap_of(blobA, aps, 99, CA_W1P + p * 512 + m * 128,
                         [[256, 2], [1, 128]])

        def w2t_ap():
            return ap_of(blobA, aps, 128, CA_W2T, [[64, 2], [1, 64]])

        def headw_ap(p):
            return ap_of(blobA, aps, 66, CA_HW + p * 256, [[128, 2], [1, 128]])

        def pw3_ap():
            return ap_of(blobA, aps, 64, CA_PW3, [[1, 2]])

        def dgt_ap(q, gi):
            return ap_of(blobB, bps, 128, CB_DGT + q * 512 + gi * 256,
                         [[128, 2], [1, 128]])

        def resw_ap(i, p, m):
            return ap_of(blobB, bps, 128, CB_RES + i * 1280 + p * 256,
                         [[128, 2], [1, 128]])

        def fusw_ap(k, m):
            return ap_of(blobB, bps, 128, CB_FUS + k * 512 + m * 128,
                         [[256, 2], [1, 128]])

        def pw1_ap(p, m):
            return ap_of(blobB, bps, 128, CB_PW1 + p * 512 + m * 128,
                         [[256, 2], [1, 128]])

        def pw2_ap():
            return ap_of(blobB, bps, 128, CB_PW2, [[64, 2], [1, 64]])

        def const_col(c0, parts=128, n=1):
            return ap_of(consts, cps, parts, c0, [[1, n]])

        base_ap = ap_of(consts, cps, 128, CC_BASE, [[2, PADQ], [1, 2]])

        # --- engine helpers ---
        rr_state = [0]

        def rr_relu(out_ap, in_ap, scale, bias_ap, force=None):
            """out = relu(scale*x + bias). scale may be const or AP.
            If zb (bias==0), can run on act or dve; else activation only.
            (GPSIMD cannot access PSUM, so pool never does these.)"""
            if not zb or force == 'act':
                nc.scalar.activation(out_ap, in_ap, AF.Relu,
                                     bias=(bias_ap if (bias_ap is not None and
                                                       not zb) else 0.0),
                                     scale=scale)
                return
            e = force if force is not None else ('act', 'dve')[rr_state[0] % 2]
            if force is None:
                rr_state[0] += 1
            if e == 'act':
                nc.scalar.activation(out_ap, in_ap, AF.Relu, bias=0.0,
                                     scale=scale)
            else:
                nc.vector.tensor_scalar(out_ap, in_ap, scale, 0.0,
                                        op0=ALU.mult, op1=ALU.max)

        def rr_copy(out_ap, in_ap, scale=None, force=None, pool_ok=False):
            engs = ('act', 'dve', 'pool') if pool_ok else ('act', 'dve')
            e = force if force is not None else engs[rr_state[0] % len(engs)]
            if force is None:
                rr_state[0] += 1
            if scale is None:
                if e == 'act':
                    nc.scalar.activation(out_ap, in_ap, AF.Copy, bias=0.0)
                elif e == 'dve':
                    nc.vector.tensor_copy(out_ap, in_ap)
                else:
                    nc.gpsimd.tensor_copy(out_ap, in_ap)
            else:
                if e == 'act':
                    nc.scalar.activation(out_ap, in_ap, AF.Copy, bias=0.0,
                                         scale=scale)
                elif e == 'dve':
                    nc.vector.tensor_scalar(out_ap, in_ap, scale, None,
                                            op0=ALU.mult)
                else:
                    nc.gpsimd.tensor_scalar(out_ap, in_ap, scale, None,
                                            op0=ALU.mult)

        # ------------ conv1 (3x3, 66->256) + conv2 (1x1, 256->64) ------------
        with tc.tile_pool(name="conv", bufs=1) as cpool, \
             tc.tile_pool(name="psumA", bufs=3, space="PSUM") as ppA, \
             tc.tile_pool(name="psumB", bufs=2, space="PSUM") as ppB, \
             tc.tile_pool(name="stage", bufs=3) as spool:
            r1 = cpool.tile([128, 2, HW], fp8)
            ps2 = {}

            def emit_conv2(g):
                h = g % 2
                if h == 0:
                    ps2[0] = ppB.tile([128, 8, 64], f32, tag="c2", name="c2")
                for cc in range(4):
                    pos0 = g * 512 + cc * 128
                    lhsT = bass.AP(tensor=r1.tensor, offset=r1.offset + pos0,
                                   ap=[r1.ap[0], [HW, 2], [1, 128]])
                    nc.tensor.matmul(ps2[0][:, h * 4 + cc, :], lhsT, w2t_ap(),
                                     start=True, stop=True, perf_mode=DR)
                if h == 1:
                    stg = spool.tile([128, 8, 64], f32, tag="stage", name="stg")
                    rr_copy(stg, ps2[0], 1.0 / (A_W * S_R1))
                    dst = bass.AP(tensor=feat_dram,
                                  offset=((g - 1) * 512) * 64,
                                  ap=[[64, 128], [8192, 8], [1, 64]])
                    deng = nc.sync if (g // 2) % 2 == 0 else nc.gpsimd
                    deng.dma_start(out=dst, in_=stg)

            for g in range(32):              # y-groups of 4 rows
                for m in range(2):
                    pg = ppA.tile([128, 4, 128], f32, tag=f"c1_{m}",
                                  name=f"c1_{m}")
                    for yy in range(4):
                        y = 4 * g + yy
                        ktaps = ((2 * y * PADW, 2), (2 * y * PADW + 1, 3),
                                 (2 * (y * PADW + 1) + 1, 2))
                        for p, (off, stride) in enumerate(ktaps):
                            rhs = bass.AP(tensor=stk_t.tensor,
                                          offset=stk_t.offset + off,
                                          ap=[stk_t.ap[0], [stride, 2],
                                              [2, 128]])
                            nc.tensor.matmul(pg[:, yy, :], w1p_ap(p, m),
                                             rhs, start=(p == 0), stop=(p == 2),
                                             perf_mode=DR)
                    rr_relu(r1[:, m, g * 512:(g + 1) * 512],
                            pg.rearrange("p a b -> p (a b)"), S_R1 / A_W,
                            const_col(CC_PB0 + m))
                if g >= 2:
                    emit_conv2(g - 2)        # skewed to avoid PE queue stall
            emit_conv2(30)
            emit_conv2(31)

        # ------------ gather + combine-transpose into ipad ------------
        with tc.tile_pool(name="snake", bufs=1) as sn:
            ipad = sn.tile([128, PADQ, PADV], fp8, tag="ipad", name="ipad")
            zall = sn.tile([128, 8, PADQ, PADV], fp8, tag="zall", name="zall")
            rsc = sn.tile([128, PADQ, 128], fp8, tag="rsc", name="rsc")

            with tc.tile_pool(name="gat", bufs=1) as gp, \
                 tc.tile_pool(name="psumG", bufs=4, space="PSUM") as ppG, \
                 tc.tile_pool(name="psumS", bufs=4, space="PSUM") as ppS:
                gta = gp.tile([128, PADQ, 128], f32, tag="gta", name="gta")
                gtb = gp.tile([128, PADQ, 128], f32, tag="gtb", name="gtb")
                g8a = gp.tile([128, PADQ, 128], fp8, tag="g8a", name="g8a")
                g8b = gp.tile([128, PADQ, 128], fp8, tag="g8b", name="g8b")
                gsrc = bass.AP(tensor=feat_dram, offset=0,
                               ap=[[64, HW - 1], [1, 128]])
                HQ = PADQ // 2
                HV = NV // 2

                def emit_gathers(hh):
                    qh = slice(hh * HQ, (hh + 1) * HQ)
                    nc.gpsimd.dma_gather(
                        gta[:, qh, :], gsrc,
                        gix_t[:, hh * HV // 16:(hh + 1) * HV // 16],
                        HV, HV, 128, elem_step=64, single_packet=False)
                    nc.gpsimd.dma_gather(
                        gtb[:, qh, :], gsrc,
                        gix_t[:, (2 + hh) * HV // 16:(3 + hh) * HV // 16],
                        HV, HV, 128, elem_step=64, single_packet=False)

                emit_gathers(0)
                nc.scalar.dma_start(out=ipad[64:66, :, :], in_=d_cpv[:, :, :])

                def emit_gather_block(qb):
                    qsl = slice(4 * qb, 4 * qb + 4)
                    rr_copy(g8a[:, qsl, :], gta[:, qsl, :], scale=S_FEAT,
                            pool_ok=True)
                    rr_copy(g8b[:, qsl, :], gtb[:, qsl, :], scale=S_FEAT,
                            pool_ok=True)
                    pg = ppG.tile([64, 4, 128], f32, tag="dg", name="dg")
                    for qq in range(4):
                        q = 4 * qb + qq
                        last = 2 if with_b2 else 1
                        for gi, gt in enumerate((g8a, g8b)):
                            lhsT = bass.AP(
                                tensor=gt.tensor,
                                offset=gt.offset + q * 128,
                                ap=[gt.ap[0], [64, 2], [1, 64]])
                            nc.tensor.matmul(pg[:, qq, :], lhsT,
                                             dgt_ap(q, gi),
                                             start=(gi == 0), stop=(gi == last),
                                             perf_mode=DR)
                        if with_b2:
                            nc.tensor.matmul(pg[:, qq, :], b2r_t[:, :],
                                             svr_t[:, q, :], start=False,
                                             stop=True)
                    rr_copy(ipad[0:64, qsl, 16:144], pg)
                    weng = nc.vector if qb % 2 == 0 else nc.gpsimd
                    weng.tensor_copy(ipad[0:66, qsl, 0:16],
                                     ipad[0:66, qsl, 128:144])
                    weng.tensor_copy(ipad[0:66, qsl, 144:160],
                                     ipad[0:66, qsl, 16:32])

                def conv_layer_qb(zo, rhs_base_fn, rhs_tensor, rhs_ap0,
                                  lhsT_fn, dil, src_zi, qb):
                    ps = ppS.tile([128, 4, 128], f32, tag="psS", name="psS")
                    for qq in range(4):
                        q = 4 * qb + qq
                        for p in range(5):
                            if p < 4:
                                off = rhs_base_fn(q) + 16 + (2 * p - 4) * dil
                                stride = dil
                            else:
                                off = rhs_base_fn(q) + 16 + 4 * dil
                                stride = 0
                            rhs = bass.AP(tensor=rhs_tensor, offset=off,
                                          ap=[rhs_ap0, [stride, 2], [1, 128]])
                            nc.tensor.matmul(ps[:, qq, :], lhsT_fn(p), rhs,
                                             start=(p == 0), stop=(p == 4),
                                             perf_mode=DR)
                    qsl = slice(4 * qb, 4 * qb + 4)
                    scale_ap = const_col(CC_LSB + 2 * zo)
                    bias_ap = const_col(CC_LSB + 2 * zo + 1)
                    if src_zi is None:
                        rr_relu(zall[:, 0, qsl, 16:144], ps, scale_ap,
                                bias_ap, force=('act' if qb != 4 else 'dve'))
                    else:
                        rr_relu(rsc[:, qsl, :], ps, scale_ap, bias_ap,
                                force=('act' if qb != 4 else 'dve'))
                        addeng = nc.gpsimd if qb in (1, 2, 3, 4) else nc.vector
                        addeng.tensor_tensor(
                            zall[:, zo, qsl, 16:144],
                            zall[:, src_zi, qsl, 16:144],
                            rsc[:, qsl, :], ALU.add)
                    weng = nc.vector if qb % 2 == 0 else nc.gpsimd
                    weng.tensor_copy(zall[:, zo, qsl, 0:16],
                                     zall[:, zo, qsl, 128:144])
                    weng.tensor_copy(zall[:, zo, qsl, 144:160],
                                     zall[:, zo, qsl, 16:32])

                ip66 = ipad[0:66, :, :]

                def head_qb(qb):
                    conv_layer_qb(0, lambda q: ip66.offset + q * PADV,
                                  ip66.tensor, ip66.ap[0], headw_ap, 1, None,
                                  qb)

                # interleave: gather blocks then head conv per poly-half
                for qb in range(NQB // 2):
                    emit_gather_block(qb)
                emit_gathers(1)
                for qb in range(NQB // 2):
                    head_qb(qb)
                for qb in range(NQB // 2, NQB):
                    emit_gather_block(qb)
                for qb in range(NQB // 2, NQB):
                    head_qb(qb)

                for i in range(NRES):
                    zi_off = zall.offset + i * PADQ * PADV
                    for qb in range(NQB):
                        conv_layer_qb(i + 1,
                                      lambda q, zi_off=zi_off: zi_off + q * PADV,
                                      zall.tensor, zall.ap[0],
                                      lambda p, i=i: resw_ap(i, p, 0),
                                      DIL[i], i, qb)

                # fusion 1x1 (1024->256) + per-poly max over V
                gmax = sn.tile([128, 2, PADQ], f32, tag="gmax", name="gmax")
                gb = sn.tile([128, 2, PADQ], fp8, tag="gb", name="gb")
                for m in range(2):
                    for qb in range(NQB):
                        ps = ppS.tile([128, 4, 128], f32, tag="psS", name="psS")
                        for qq in range(4):
                            q = 4 * qb + qq
                            for k in range(4):
                                off = zall.offset + (2 * k * PADQ + q) * PADV + 16
                                rhs = bass.AP(tensor=zall.tensor, offset=off,
                                              ap=[zall.ap[0],
                                                  [PADQ * PADV, 2], [1, 128]])
                                nc.tensor.matmul(ps[:, qq, :],
                                                 fusw_ap(k, m), rhs,
                                                 start=(k == 0), stop=(k == 3),
                                                 perf_mode=DR)
                        nc.vector.tensor_reduce(gmax[:, m, 4 * qb:4 * qb + 4],
                                                ps, axis=mybir.AxisListType.X,
                                                op=ALU.max)
                    nc.vector.tensor_scalar(gb[:, m, :], gmax[:, m, :],
                                            S_GB / (A_W * S_Z),
                                            const_col(CC_FUSC + m),
                                            op0=ALU.mult, op1=ALU.add)

                # pred1: 1280 -> 256 relu
                h1 = sn.tile([128, 2, PADQ, 128], fp8, tag="h1", name="h1")
                for m in range(2):
                    for qb in range(NQB):
                        ps = ppS.tile([128, 4, 128], f32, tag="psS", name="psS")
                        for qq in range(4):
                            q = 4 * qb + qq
                            for k in range(4):
                                off = zall.offset + (2 * k * PADQ + q) * PADV + 16
                                rhs = bass.AP(tensor=zall.tensor, offset=off,
                                              ap=[zall.ap[0],
                                                  [PADQ * PADV, 2], [1, 128]])
                                nc.tensor.matmul(ps[:, qq, :],
                                                 pw1_ap(k + 1, m), rhs,
                                                 start=(k == 0), stop=False,
                                                 perf_mode=DR)
                            rhs0 = bass.AP(tensor=gb.tensor,
                                           offset=gb.offset + q,
                                           ap=[gb.ap[0], [PADQ, 2], [0, 128]])
                            nc.tensor.matmul(ps[:, qq, :], pw1_ap(0, m),
                                             rhs0, start=False, stop=True,
                                             perf_mode=DR)
                        rr_relu(h1[:, m, 4 * qb:4 * qb + 4, :], ps,
                                S_H1 / (A_W * S_Z), const_col(CC_PB1 + m))

            # pred2 + pred3
            with tc.tile_pool(name="psumT", bufs=3, space="PSUM") as ppT:
                h2 = sn.tile([64, PADQ, 128], fp8, tag="h2", name="h2")
                for qb in range(NQB):
                    ps = ppT.tile([64, 4, 128], f32, tag="psT", name="psT")
                    for qq in range(4):
                        q = 4 * qb + qq
                        rhs = bass.AP(tensor=h1.tensor,
                                      offset=h1.offset + q * 128,
                                      ap=[h1.ap[0], [PADQ * 128, 2], [1, 128]])
                        nc.tensor.matmul(ps[:, qq, :], pw2_ap(), rhs,
                                         start=True, stop=True, perf_mode=DR)
                    rr_relu(h2[:, 4 * qb:4 * qb + 4, :], ps,
                            S_H2 / (A_W * S_H1), const_col(CC_PB2, parts=64))

                ps3 = ppT.tile([128, PADQ, 2], f32, tag="psT3", name="psT3",
                               bufs=1)
                for q in range(PADQ):
                    nc.tensor.matmul(ps3[:, q, :], h2[:, q, :], pw3_ap(),
                                     start=True, stop=True)
                o_f = sn.tile([128, PADQ, 2], f32, tag="o_f", name="o_f")
                nc.vector.tensor_scalar(o_f, ps3, 1.0 / (A_W * S_H2), None,
                                        op0=ALU.mult)
                o_t = sn.tile([128, PADQ, 2], f32, tag="o_t", name="o_t")
                nc.vector.tensor_tensor(o_t, o_f, base_ap, ALU.add)
                nc.sync.dma_start(out=d_out[:, :, :], in_=o_t)

    nc.compile()
    return nc


_NC_CACHE = {}


def _get_nc_key(P, with_b2, zb):
    key = (P, with_b2, zb)
    if key not in _NC_CACHE:
        _NC_CACHE[key] = build_nc(P, with_b2, zb)
    return _NC_CACHE[key]


def _get_nc(P):
    """test.py compatibility: default flags for the standard input set."""
    return _get_nc_key(P, False, True)


def _host_prep(inputs, P, counts, order, offs):
    """Build per-core in_maps. Returns (in_maps, with_b2, zb)."""
    PADQ = -(-P // 4) * 4
    NV = PADQ * 128
    cnn = np.asarray(inputs["cnn_feature"], np.float32)
    ipoly = np.asarray(inputs["i_it_poly"], np.float32)
    cpoly = np.asarray(inputs["c_it_poly"], np.float32)
    w1 = np.asarray(inputs["proj_w1"], np.float32)
    pb0 = np.asarray(inputs["proj_b1"], np.float32)
    w2 = np.asarray(inputs["proj_w2"], np.float32)[:, :, 0, 0]  # [64, 256]
    b2 = np.asarray(inputs["proj_b2"], np.float32)
    head_w = np.asarray(inputs["head_w"], np.float32)   # [128, 66, 9]
    head_b = np.asarray(inputs["head_b"], np.float32)
    head_g = np.asarray(inputs["head_g"], np.float32)
    head_bt = np.asarray(inputs["head_bt"], np.float32)
    res_w = np.asarray(inputs["res_w"], np.float32)     # [7, 128, 128, 9]
    res_b = np.asarray(inputs["res_b"], np.float32)
    res_g = np.asarray(inputs["res_g"], np.float32)
    res_bt = np.asarray(inputs["res_bt"], np.float32)
    fus_w = np.asarray(inputs["fus_w"], np.float32)     # [256, 1024]
    fus_b = np.asarray(inputs["fus_b"], np.float32)
    pw1 = np.asarray(inputs["pw1"], np.float32)         # [256, 1280]
    pb1 = np.asarray(inputs["pb1"], np.float32)
    pw2 = np.asarray(inputs["pw2"], np.float32)         # [64, 256]
    pb2 = np.asarray(inputs["pb2"], np.float32)
    pw3 = np.asarray(inputs["pw3"], np.float32)         # [2, 64]
    pb3 = np.asarray(inputs["pb3"], np.float32)

    assert (head_g > 0).all() and (res_g > 0).all(), "bn fold requires g>0"
    with_b2 = bool(np.any(b2 != 0))

    # w1p [99, 3pairs, 2kt, 2m, 128]
    w1p = np.zeros((99, 3, 2, 2, 128), np.float32)
    pair_src = [((0, 0), (0, 1)), ((1, 0), (0, 2)), ((1, 1), (1, 2))]
    for p, pr in enumerate(pair_src):
        for kt, (blk, kw) in enumerate(pr):
            rm = ROWMAP_A if blk == 0 else ROWMAP_B
            for r in range(99):
                ch, kh = rm[r]
                for m in range(2):
                    w1p[r, p, kt, m, :] = A_W * w1[m * 128:(m + 1) * 128,
                                                   ch, kh, kw]
    pb0s = (S_R1 * pb0).reshape(2, 128).T.copy()

    w2t = np.zeros((128, 2, 64), np.float32)
    for kt in range(2):
        w2t[:, kt, :] = A_W * w2[:, kt * 128:(kt + 1) * 128].T

    # ---- grid-sample host math ----
    ix = ipoly[..., 0] - np.float32(0.5)
    iy = ipoly[..., 1] - np.float32(0.5)
    x0 = np.floor(ix); y0 = np.floor(iy)
    wx = (ix - x0).astype(np.float32); wy = (iy - y0).astype(np.float32)
    x0i = x0.astype(np.int64); y0i = y0.astype(np.int64)

    swap_x = x0i < 0
    vx0 = (x0i >= 0) & (x0i < W)
    vx1 = (x0i + 1 >= 0) & (x0i + 1 < W)

    def slot_weights(yi):
        vy = (yi >= 0) & (yi < H)
        w_s0 = (1 - wx) * vx0 * vy
        w_s1 = wx * vx1 * vy
        w_s0 = np.where(swap_x, wx * vx1 * vy, w_s0)
        w_s1 = np.where(swap_x, 0.0, w_s1)
        return w_s0.astype(np.float32), w_s1.astype(np.float32)

    x0c = np.clip(x0i, 0, W - 2)
    y0c = np.clip(y0i, 0, H - 1)
    y1c = np.clip(y0i + 1, 0, H - 1)
    idxA = (y0c * W + x0c).astype(np.int64)          # [NP, V]
    idxB = (y1c * W + x0c).astype(np.int64)
    wA0, wA1 = slot_weights(y0i)
    wB0, wB1 = slot_weights(y0i + 1)
    wA0 *= (1 - wy); wA1 *= (1 - wy)
    wB0 *= wy; wB1 *= wy
    s_v = wA0 + wA1 + wB0 + wB1

    # ---- snake weights (bn + scale folds) ----
    headw = np.zeros((66, 5, 2, 128), np.float32)
    hw9 = head_w.transpose(1, 2, 0)                  # [66, 9, 128]
    for t in range(9):
        p, kt = t // 2, t % 2
        headw[0:64, p, kt, :] = A_W * hw9[0:64, t, :]
        headw[64:66, p, kt, :] = A_W * S_FEAT * hw9[64:66, t, :]
    lsb = np.zeros((128, 8, 2), np.float32)
    lsb[:, 0, 0] = head_g / A_W
    lsb[:, 0, 1] = S_Z * head_g * head_b
    C = np.zeros((8, 128), np.float32)               # C_i = sum_{j<=i} bt_j
    C[0] = head_bt
    for i in range(NRES):
        C[i + 1] = C[i] + res_bt[i]

    resw = np.zeros((128, 7, 5, 2, 128), np.float32)
    for i in range(NRES):
        rw = res_w[i].transpose(1, 2, 0)             # [128 in, 9, 128 out]
        for t in range(9):
            p, kt = t // 2, t % 2
            resw[:, i, p, kt, :] = A_W * rw[:, t, :]
        bprime = res_b[i] + res_w[i].sum(axis=2) @ C[i]
        lsb[:, i + 1, 0] = res_g[i] / A_W
        lsb[:, i + 1, 1] = S_Z * res_g[i] * bprime

    fw8 = fus_w.reshape(256, 8, 128)
    fusw = np.zeros((128, 4, 2, 2, 128), np.float32)
    for k in range(4):
        for kt in range(2):
            for m in range(2):
                fusw[:, k, kt, m, :] = A_W * fw8[m * 128:(m + 1) * 128,
                                                 2 * k + kt].T
    fusconst = fus_b + np.einsum('ojc,jc->o', fw8, C)
    fusc = (S_GB * fusconst).reshape(2, 128).T.copy()

    pw1r = pw1.reshape(256, 10, 128)
    pw1p = np.zeros((128, 5, 2, 2, 128), np.float32)
    for m in range(2):
        for kt in range(2):
            pw1p[:, 0, kt, m, :] = (A_W * S_Z / S_GB) * \
                pw1r[m * 128:(m + 1) * 128, kt].T
        for k in range(4):
            for kt in range(2):
                pw1p[:, k + 1, kt, m, :] = A_W * \
                    pw1r[m * 128:(m + 1) * 128, 2 + 2 * k + kt].T
    pb1prime = pb1 + np.einsum('ojc,jc->o', pw1r[:, 2:], C)
    pb1s = (S_H1 * pb1prime).reshape(2, 128).T.copy()

    pw2t = np.zeros((128, 2, 64), np.float32)
    for kt in range(2):
        pw2t[:, kt, :] = A_W * pw2[:, kt * 128:(kt + 1) * 128].T
    pb2s = (S_H2 * pb2).reshape(64, 1)
    pw3t = A_W * pw3.T                                # [64, 2]

    zb = (not np.any(pb0)) and (not np.any(lsb[:, :, 1])) \
        and (not np.any(pb1s)) and (not np.any(pb2s))

    # ---- pack blobs ----
    CA = 2946
    blobA = np.zeros((128, CA), F8)
    blobA[0:99, 0:1536] = _f8(w1p).reshape(99, -1)
    blobA[:, 1536:1664] = _f8(w2t).reshape(128, -1)
    blobA[0:66, 1664:2944] = _f8(headw).reshape(66, -1)
    blobA[0:64, 2944:2946] = _f8(pw3t)

    CB_RES = PADQ * 512
    CB_FUS = CB_RES + 8960
    CB_PW1 = CB_FUS + 2048
    CB_PW2 = CB_PW1 + 2560
    CB = CB_PW2 + 128
    blobB_shared = np.zeros((128, CB - CB_RES), F8)
    blobB_shared[:, 0:8960] = _f8(resw).reshape(128, -1)
    blobB_shared[:, 8960:8960 + 2048] = _f8(fusw).reshape(128, -1)
    blobB_shared[:, 11008:11008 + 2560] = _f8(pw1p).reshape(128, -1)
    pw2pad = np.zeros((128, 2, 64), np.float32)
    pw2pad[:, :, :] = pw2t[:, :, 0:64]
    blobB_shared[:, 13568:13696] = _f8(pw2pad).reshape(128, -1)

    CC = 24 + PADQ * 2
    consts_shared = np.zeros((128, CC), np.float32)
    consts_shared[:, 0:2] = pb0s
    consts_shared[:, 2:18] = lsb.reshape(128, -1)
    consts_shared[:, 18:20] = fusc
    consts_shared[:, 20:22] = pb1s
    consts_shared[0:64, 22:23] = pb2s

    shared = {"blobA": blobA}
    if with_b2:
        shared["b2r"] = _f8(S_FEAT * b2.reshape(1, 64))

    in_maps = []
    for c in range(N_CORES):
        img = cnn[c]
        img_pad = np.zeros((C_IN, PADW, PADW), np.float32)
        img_pad[:, 1:129, 1:129] = img
        flatf = _f8(img_pad.reshape(C_IN, PIMG)).astype(np.float32)
        stk = np.zeros((99, PIMG, 2), np.float32)
        for r in range(99):
            ch, kh = ROWMAP_A[r]
            ln = PIMG - kh * PADW
            stk[r, :ln, 0] = flatf[ch, kh * PADW:]
            ch, kh = ROWMAP_B[r]
            ln = PIMG - kh * PADW
            stk[r, :ln, 1] = flatf[ch, kh * PADW:]
        stk = stk.reshape(99, 2 * PIMG)

        own = order[offs[c]:offs[c + 1]]
        nown = len(own)
        gixa = np.zeros(NV, np.int64)
        gixb = np.zeros(NV, np.int64)
        dgt = np.zeros((128, PADQ, 4, 128), np.float32)
        ar = np.arange(128)
        for qi, poly in enumerate(own):
            gixa[qi * 128:(qi + 1) * 128] = idxA[poly]
            gixb[qi * 128:(qi + 1) * 128] = idxB[poly]
            dgt[ar, qi, 0, ar] = wA0[poly]
            dgt[ar, qi, 1, ar] = wA1[poly]
            dgt[ar, qi, 2, ar] = wB0[poly]
            dgt[ar, qi, 3, ar] = wB1[poly]

        cpv = np.zeros((2, PADQ, 160), np.float32)
        if nown:
            cc = (cpoly[own] * RO).transpose(2, 0, 1)     # [2, nown, 128]
            cpv[:, :nown, 16:144] = cc
            cpv[:, :nown, 0:16] = cc[:, :, 112:128]
            cpv[:, :nown, 144:160] = cc[:, :, 0:16]

        base = np.zeros((128, PADQ, 2), np.float32)
        if nown:
            base[:, :nown, :] = (ipoly[own] * RO + pb3[None, None, :]) \
                .transpose(1, 0, 2)

        blobB = np.zeros((128, CB), F8)
        blobB[:, 0:CB_RES] = _f8(dgt).reshape(128, -1)
        blobB[:, CB_RES:] = blobB_shared
        consts = consts_shared.copy()
        consts[:, 24:] = base.reshape(128, -1)
        gix = np.concatenate([pack16(gixa, NV // 16),
                              pack16(gixb, NV // 16)], axis=1)
        m = {
            "stk": _f8(stk), "blobB": blobB, "consts": consts,
            "gix": gix, "cpv": _f8(cpv),
        }
        if with_b2:
            svr = np.zeros((1, PADQ, 128), np.float32)
            svr[0, :nown, :] = s_v[own]
            m["svr"] = _f8(svr)
        m.update(shared)
        in_maps.append(m)
    return in_maps, with_b2, zb


def kernel(**inputs):
    ind = np.asarray(inputs["ind"]).astype(np.int64)
    counts = np.bincount(ind, minlength=N_CORES)
    P = int(counts.max())
    order = np.argsort(ind, kind="stable")
    offs = np.concatenate([[0], np.cumsum(counts)])

    in_maps, with_b2, zb = _host_prep(inputs, P, counts, order, offs)
    nc = _get_nc_key(P, with_b2, zb)
    res = None
    last_err = None
    for _attempt in range(3):
        try:
            res = run_bass_kernel_spmd(nc, in_maps, list(range(N_CORES)))
            break
        except Exception as e:  # rare transient device error; retry
            last_err = e
    if res is None:
        raise last_err

    out = np.zeros((NP, V, 2), np.float32)
    for c in range(N_CORES):
        oc = res.results[c]["out"]  # [128v, PADQ, 2]
        own = order[offs[c]:offs[c + 1]]
        for q, opoly in enumerate(own):
            out[opoly] = oc[:, q, :]
    return out


# revision 18
# speedup vs baseline: 1.3263x; 1.0117x over previous
"""Trainium2 Bass kernel for nn_Evolution_26697516712465 (deep-snake GNN).

Self-contained: takes FULL inputs, shards batch across 8 NeuronCores internally
(one image per core; each core runs the snake for the polys of its own image),
returns FULL output [128, 128, 2] fp32.

fp8e4 (e4m3) DoubleRow matmuls throughout (2 contraction rows/cycle), weights
pre-scaled by 64 into fp8 normal range, activations carried at power-of-2
scales; bilinear grid-sample folded into PE "diagonal" matmuls on gathered
corner row-pairs; eval-mode bn folded into weights/biases host-side.
"""
import numpy as np
import ml_dtypes
from contextlib import ExitStack

import concourse.bass as bass
import concourse.bacc as bacc
import concourse.mybir as mybir
import concourse.tile as tile
from concourse.library_config import mlp as mlp_lib
from concourse.bass_utils import run_bass_kernel_spmd

N_CORES = 8
B, C_IN, H, W = 8, 66, 128, 128
NP, V = 128, 128
RO = 4.0
DIL = (1, 1, 1, 2, 2, 4, 4)
NRES = 7
HW = H * W          # 16384
PADW = W + 2        # 130
PIMG = PADW * PADW  # 16900
PADV = 160          # 16 + 128 + 16 circular pad

f32 = mybir.dt.float32
f32r = mybir.dt.float32r
fp8 = mybir.dt.float8e4
i16 = mybir.dt.int16
AF = mybir.ActivationFunctionType
ALU = mybir.AluOpType
DR = mybir.MatmulPerfMode.DoubleRow

F8 = ml_dtypes.float8_e4m3

# activation/weight scales (powers of 2)
A_W = 64.0          # weight scale
S_R1 = 8.0          # conv1 relu out
S_FEAT = 32.0       # ipad feat rows (folded into diag weights)
S_Z = 32.0          # snake states
S_GB = 128.0        # fusion global feature
S_H1 = 128.0
S_H2 = 512.0

# conv1 stack row maps: blkA = 66ch kh0 + 33ch kh1; blkB = 33ch kh1 + 66ch kh2
ROWMAP_A = [(r, 0) if r < 66 else (r - 66, 1) for r in range(99)]
ROWMAP_B = [(r + 33, 1) if r < 33 else (r - 33, 2) for r in range(99)]


def _f8(x):
    return np.clip(np.asarray(x, np.float32), -240.0, 240.0).astype(F8)


def pack16(idx_flat, cols):
    tab = np.zeros((16, cols), np.int16)
    n = len(idx_flat)
    tab[np.arange(n) % 16, np.arange(n) // 16] = idx_flat.astype(np.int16)
    return np.tile(tab, (8, 1))


def build_nc(P, with_b2, zb):
    """Build the SPMD Bass program. P = max polys per image; zb = all relu
    biases are zero (allows relu on DVE/Pool engines)."""
    nc = bacc.Bacc("TRN2", target_bir_lowering=False, debug=False)
    PADQ = -(-P // 4) * 4
    NQB = PADQ // 4
    NV = PADQ * 128

    # ---------------- inputs ----------------
    # blobA: w1p | w2t | headw | pw3 (fp8, conv+early weights)
    CA_W1P, CA_W2T, CA_HW, CA_PW3 = 0, 1536, 1664, 2944
    CA = 2946
    # blobB: dgt | resw | fusw | pw1 | pw2 (fp8, late weights)
    CB_DGT = 0
    CB_RES = PADQ * 512
    CB_FUS = CB_RES + 8960
    CB_PW1 = CB_FUS + 2048
    CB_PW2 = CB_PW1 + 2560
    CB = CB_PW2 + 128
    # consts (f32): pb0 | lsb | fusc | pb1 | pb2 | base
    CC_PB0, CC_LSB, CC_FUSC, CC_PB1, CC_PB2, CC_BASE = 0, 2, 18, 20, 22, 24
    CC = 24 + PADQ * 2
    d_stk = nc.declare_dram_parameter("stk", [99, 2 * PIMG], fp8, isOutput=False)
    d_blobA = nc.declare_dram_parameter("blobA", [128, CA], fp8, isOutput=False)
    d_blobB = nc.declare_dram_parameter("blobB", [128, CB], fp8, isOutput=False)
    d_consts = nc.declare_dram_parameter("consts", [128, CC], f32, isOutput=False)
    d_gix = nc.declare_dram_parameter("gix", [128, 2 * (NV // 16)], i16, isOutput=False)
    d_cpv = nc.declare_dram_parameter("cpv", [2, PADQ, 160], fp8, isOutput=False)
    if with_b2:
        d_b2r = nc.declare_dram_parameter("b2r", [1, 64], fp8, isOutput=False)
        d_svr = nc.declare_dram_parameter("svr", [1, PADQ, 128], fp8, isOutput=False)
    d_out = nc.declare_dram_parameter("out", [128, PADQ, 2], f32, isOutput=True)

    feat_dram = nc.dram_tensor("feat_dram", [HW, 64], f32)

    with tile.TileContext(nc, num_cores=N_CORES) as tc, ExitStack() as top:
        wpool = top.enter_context(tc.tile_pool(name="weights", bufs=1))
        # SP queue: blobA, consts, gix (small, early)
        blobA = wpool.tile([128, CA], fp8)
        nc.sync.dma_start(out=blobA[:, 0:CA_W2T], in_=d_blobA[:, 0:CA_W2T])
        nc.sync.dma_start(out=blobA[:, CA_W2T:], in_=d_blobA[:, CA_W2T:])
        if with_b2:
            b2r_t = wpool.tile([1, 64], fp8)
            nc.sync.dma_start(out=b2r_t, in_=d_b2r[:, :])
            svr_t = wpool.tile([1, PADQ, 128], fp8)
            nc.sync.dma_start(out=svr_t, in_=d_svr[:, :, :])

        nc.gpsimd.load_library(mlp_lib)

        # Pool DMA queue: stk bands, then blobB chunks (dgt first)
        stk_t = wpool.tile([99, 2 * PIMG], fp8)
        CHK = 2 * 16 * PADW
        for bb in range(8):
            nc.sync.dma_start(out=stk_t[:, bb * CHK:(bb + 1) * CHK],
                              in_=d_stk[:, bb * CHK:(bb + 1) * CHK])
        consts = wpool.tile([128, CC], f32)
        nc.sync.dma_start(out=consts, in_=d_consts[:, :])
        gix_t = wpool.tile([128, 2 * (NV // 16)], i16)
        nc.sync.dma_start(out=gix_t, in_=d_gix[:, :])
        blobB = wpool.tile([128, CB], fp8)
        nc.gpsimd.dma_start(out=blobB[:, CB_DGT:CB_RES],
                            in_=d_blobB[:, CB_DGT:CB_RES])
        for i in range(NRES):
            c0 = CB_RES + i * 1280
            nc.gpsimd.dma_start(out=blobB[:, c0:c0 + 1280],
                                in_=d_blobB[:, c0:c0 + 1280])
        nc.gpsimd.dma_start(out=blobB[:, CB_FUS:CB],
                            in_=d_blobB[:, CB_FUS:CB])

        aps = blobA.ap[0][0]
        bps = blobB.ap[0][0]
        cps = consts.ap[0][0]

        def ap_of(blob, pstride, parts, col0, dims):
            return bass.AP(tensor=blob.tensor, offset=blob.offset + col0,
                           ap=[[pstride, parts]] + dims)

        def w1p_ap(p, m):
            return ap_of(blobA, aps, 99, CA_W1P + p * 512 + m * 128,
                         [[256, 2], [1, 128]])

        def w2t_ap():
            return ap_of(blobA, aps, 128, CA_W2T, [[64, 2], [1, 64]])

        def headw_ap(p):
            return ap_of(blobA, aps, 66, CA_HW + p * 256, [[128, 2], [1, 128]])

        def pw3_ap():
            return ap_of(blobA, aps, 64, CA_PW3, [[1, 2]])

        def dgt_ap(q, gi):
            return ap_of(blobB, bps, 128, CB_DGT + q * 512 + gi * 256,
                         [[128, 2], [1, 128]])

        def resw_ap(i, p, m):
            return ap_of(blobB, bps, 128, CB_RES + i * 1280 + p * 256,
                         [[128, 2], [1, 128]])

        def fusw_ap(k, m):
            return ap_of(blobB, bps, 128, CB_FUS + k * 512 + m * 128,
                         [[256, 2], [1, 128]])

        def pw1_ap(p, m):
            return ap_of(blobB, bps, 128, CB_PW1 + p * 512 + m * 128,
                         [[256, 2], [1, 128]])

        def pw2_ap():
            return ap_of(blobB, bps, 128, CB_PW2, [[64, 2], [1, 64]])

        def const_col(c0, parts=128, n=1):
            return ap_of(consts, cps, parts, c0, [[1, n]])

        base_ap = ap_of(consts, cps, 128, CC_BASE, [[2, PADQ], [1, 2]])

        # --- engine helpers ---
        rr_state = [0]

        def rr_relu(out_ap, in_ap, scale, bias_ap, force=None):
            """out = relu(scale*x + bias). scale may be const or AP.
            If zb (bias==0), can run on act or dve; else activation only.
            (GPSIMD cannot access PSUM, so pool never does these.)"""
            if not zb or force == 'act':
                nc.scalar.activation(out_ap, in_ap, AF.Relu,
                                     bias=(bias_ap if (bias_ap is not None and
                                                       not zb) else 0.0),
                                     scale=scale)
                return
            e = force if force is not None else ('act', 'dve')[rr_state[0] % 2]
            if force is None:
                rr_state[0] += 1
            if e == 'act':
                nc.scalar.activation(out_ap, in_ap, AF.Relu, bias=0.0,
                                     scale=scale)
            else:
                nc.vector.tensor_scalar(out_ap, in_ap, scale, 0.0,
                                        op0=ALU.mult, op1=ALU.max)

        def rr_copy(out_ap, in_ap, scale=None, force=None, pool_ok=False):
            engs = ('act', 'dve', 'pool') if pool_ok else ('act', 'dve')
            e = force if force is not None else engs[rr_state[0] % len(engs)]
            if force is None:
                rr_state[0] += 1
            if scale is None:
                if e == 'act':
                    nc.scalar.activation(out_ap, in_ap, AF.Copy, bias=0.0)
                elif e == 'dve':
                    nc.vector.tensor_copy(out_ap, in_ap)
                else:
                    nc.gpsimd.tensor_copy(out_ap, in_ap)
            else:
                if e == 'act':
                    nc.scalar.activation(out_ap, in_ap, AF.Copy, bias=0.0,
                                         scale=scale)
                elif e == 'dve':
                    nc.vector.tensor_scalar(out_ap, in_ap, scale, None,
                                            op0=ALU.mult)
                else:
                    nc.gpsimd.tensor_scalar(out_ap, in_ap, scale, None,
                                            op0=ALU.mult)

        # ------------ conv1 (3x3, 66->256) + conv2 (1x1, 256->64) ------------
        with tc.tile_pool(name="conv", bufs=1) as cpool, \
             tc.tile_pool(name="psumA", bufs=2, space="PSUM") as ppA, \
             tc.tile_pool(name="psumB", bufs=4, space="PSUM") as ppB, \
             tc.tile_pool(name="stage", bufs=3) as spool:
            r1 = cpool.tile([128, 2, HW], fp8)
            ps2 = {}

            def emit_conv2(g):
                h = g % 2
                if h == 0:
                    ps2[0] = ppB.tile([128, 8, 64], f32, tag="c2", name="c2")
                for cc in range(4):
                    pos0 = g * 512 + cc * 128
                    lhsT = bass.AP(tensor=r1.tensor, offset=r1.offset + pos0,
                                   ap=[r1.ap[0], [HW, 2], [1, 128]])
                    nc.tensor.matmul(ps2[0][:, h * 4 + cc, :], lhsT, w2t_ap(),
                                     start=True, stop=True, perf_mode=DR)
                if h == 1:
                    stg = spool.tile([128, 8, 64], f32, tag="stage", name="stg")
                    rr_copy(stg, ps2[0], 1.0 / (A_W * S_R1))
                    dst = bass.AP(tensor=feat_dram,
                                  offset=((g - 1) * 512) * 64,
                                  ap=[[64, 128], [8192, 8], [1, 64]])
                    deng = nc.sync if (g // 2) % 2 == 0 else nc.gpsimd
                    deng.dma_start(out=dst, in_=stg)

            for g in range(32):              # y-groups of 4 rows
                for m in range(2):
                    pg = ppA.tile([128, 4, 128], f32, tag=f"c1_{m}",
                                  name=f"c1_{m}")
                    for yy in range(4):
                        y = 4 * g + yy
                        ktaps = ((2 * y * PADW, 2), (2 * y * PADW + 1, 3),
                                 (2 * (y * PADW + 1) + 1, 2))
                        for p, (off, stride) in enumerate(ktaps):
                            rhs = bass.AP(tensor=stk_t.tensor,
                                          offset=stk_t.offset + off,
                                          ap=[stk_t.ap[0], [stride, 2],
                                              [2, 128]])
                            nc.tensor.matmul(pg[:, yy, :], w1p_ap(p, m),
                                             rhs, start=(p == 0), stop=(p == 2),
                                             perf_mode=DR)
                    rr_relu(r1[:, m, g * 512:(g + 1) * 512],
                            pg.rearrange("p a b -> p (a b)"), S_R1 / A_W,
                            const_col(CC_PB0 + m))
                if g >= 2:
                    emit_conv2(g - 2)        # skewed to avoid PE queue stall
            emit_conv2(30)
            emit_conv2(31)

        # ------------ gather + combine-transpose into ipad ------------
        with tc.tile_pool(name="snake", bufs=1) as sn:
            ipad = sn.tile([128, PADQ, PADV], fp8, tag="ipad", name="ipad")
            zall = sn.tile([128, 8, PADQ, PADV], fp8, tag="zall", name="zall")
            rsc = sn.tile([128, PADQ, 128], fp8, tag="rsc", name="rsc")

            with tc.tile_pool(name="gat", bufs=1) as gp, \
                 tc.tile_pool(name="psumG", bufs=4, space="PSUM") as ppG, \
                 tc.tile_pool(name="psumS", bufs=4, space="PSUM") as ppS:
                gta = gp.tile([128, PADQ, 128], f32, tag="gta", name="gta")
                gtb = gp.tile([128, PADQ, 128], f32, tag="gtb", name="gtb")
                g8a = gp.tile([128, PADQ, 128], fp8, tag="g8a", name="g8a")
                g8b = gp.tile([128, PADQ, 128], fp8, tag="g8b", name="g8b")
                gsrc = bass.AP(tensor=feat_dram, offset=0,
                               ap=[[64, HW - 1], [1, 128]])
                HQ = PADQ // 2
                HV = NV // 2

                def emit_gathers(hh):
                    qh = slice(hh * HQ, (hh + 1) * HQ)
                    nc.gpsimd.dma_gather(
                        gta[:, qh, :], gsrc,
                        gix_t[:, hh * HV // 16:(hh + 1) * HV // 16],
                        HV, HV, 128, elem_step=64, single_packet=False)
                    nc.gpsimd.dma_gather(
                        gtb[:, qh, :], gsrc,
                        gix_t[:, (2 + hh) * HV // 16:(3 + hh) * HV // 16],
                        HV, HV, 128, elem_step=64, single_packet=False)

                emit_gathers(0)
                nc.scalar.dma_start(out=ipad[64:66, :, :], in_=d_cpv[:, :, :])

                def emit_gather_block(qb):
                    qsl = slice(4 * qb, 4 * qb + 4)
                    rr_copy(g8a[:, qsl, :], gta[:, qsl, :], scale=S_FEAT,
                            pool_ok=True)
                    rr_copy(g8b[:, qsl, :], gtb[:, qsl, :], scale=S_FEAT,
                            pool_ok=True)
                    pg = ppG.tile([64, 4, 128], f32, tag="dg", name="dg")
                    for qq in range(4):
                        q = 4 * qb + qq
                        last = 2 if with_b2 else 1
                        for gi, gt in enumerate((g8a, g8b)):
                            lhsT = bass.AP(
                                tensor=gt.tensor,
                                offset=gt.offset + q * 128,
                                ap=[gt.ap[0], [64, 2], [1, 64]])
                            nc.tensor.matmul(pg[:, qq, :], lhsT,
                                             dgt_ap(q, gi),
                                             start=(gi == 0), stop=(gi == last),
                                             perf_mode=DR)
                        if with_b2:
                            nc.tensor.matmul(pg[:, qq, :], b2r_t[:, :],
                                             svr_t[:, q, :], start=False,
                                             stop=True)
                    rr_copy(ipad[0:64, qsl, 16:144], pg)
                    weng = nc.vector if qb % 2 == 0 else nc.gpsimd
                    weng.tensor_copy(ipad[0:66, qsl, 0:16],
                                     ipad[0:66, qsl, 128:144])
                    weng.tensor_copy(ipad[0:66, qsl, 144:160],
                                     ipad[0:66, qsl, 16:32])

                def conv_layer_qb(zo, rhs_base_fn, rhs_tensor, rhs_ap0,
                                  lhsT_fn, dil, src_zi, qb):
                    ps = ppS.tile([128, 4, 128], f32, tag="psS", name="psS")
                    for qq in range(4):
                        q = 4 * qb + qq
                        for p in range(5):
                            if p < 4:
                                off = rhs_base_fn(q) + 16 + (2 * p - 4) * dil
                                stride = dil
                            else:
                                off = rhs_base_fn(q) + 16 + 4 * dil
                                stride = 0
                            rhs = bass.AP(tensor=rhs_tensor, offset=off,
                                          ap=[rhs_ap0, [stride, 2], [1, 128]])
                            nc.tensor.matmul(ps[:, qq, :], lhsT_fn(p), rhs,
                                             start=(p == 0), stop=(p == 4),
                                             perf_mode=DR)
                    qsl = slice(4 * qb, 4 * qb + 4)
                    scale_ap = const_col(CC_LSB + 2 * zo)
                    bias_ap = const_col(CC_LSB + 2 * zo + 1)
                    if src_zi is None:
                        rr_relu(zall[:, 0, qsl, 16:144], ps, scale_ap,
                                bias_ap, force=('act' if qb != 4 else 'dve'))
                    else:
                        rr_relu(rsc[:, qsl, :], ps, scale_ap, bias_ap,
                                force=('act' if qb != 4 else 'dve'))
                        addeng = nc.gpsimd if qb in (1, 2, 3, 4) else nc.vector
                        addeng.tensor_tensor(
                            zall[:, zo, qsl, 16:144],
                            zall[:, src_zi, qsl, 16:144],
                            rsc[:, qsl, :], ALU.add)
                    weng = nc.vector if qb % 2 == 0 else nc.gpsimd
                    weng.tensor_copy(zall[:, zo, qsl, 0:16],
                                     zall[:, zo, qsl, 128:144])
                    weng.tensor_copy(zall[:, zo, qsl, 144:160],
                                     zall[:, zo, qsl, 16:32])

                ip66 = ipad[0:66, :, :]

                def head_qb(qb):
                    conv_layer_qb(0, lambda q: ip66.offset + q * PADV,
                                  ip66.tensor, ip66.ap[0], headw_ap, 1, None,
                                  qb)

                # interleave: gather blocks then head conv per poly-half
                for qb in range(NQB // 2):
                    emit_gather_block(qb)
                emit_gathers(1)
                for qb in range(NQB // 2):
                    head_qb(qb)
                for qb in range(NQB // 2, NQB):
                    emit_gather_block(qb)
                for qb in range(NQB // 2, NQB):
                    head_qb(qb)

                for i in range(NRES):
                    zi_off = zall.offset + i * PADQ * PADV
                    for qb in range(NQB):
                        conv_layer_qb(i + 1,
                                      lambda q, zi_off=zi_off: zi_off + q * PADV,
                                      zall.tensor, zall.ap[0],
                                      lambda p, i=i: resw_ap(i, p, 0),
                                      DIL[i], i, qb)

                # fusion 1x1 (1024->256) + per-poly max over V
                gmax = sn.tile([128, 2, PADQ], f32, tag="gmax", name="gmax")
                gb = sn.tile([128, 2, PADQ], fp8, tag="gb", name="gb")
                for m in range(2):
                    for qb in range(NQB):
                        ps = ppS.tile([128, 4, 128], f32, tag="psS", name="psS")
                        for qq in range(4):
                            q = 4 * qb + qq
                            for k in range(4):
                                off = zall.offset + (2 * k * PADQ + q) * PADV + 16
                                rhs = bass.AP(tensor=zall.tensor, offset=off,
                                              ap=[zall.ap[0],
                                                  [PADQ * PADV, 2], [1, 128]])
                                nc.tensor.matmul(ps[:, qq, :],
                                                 fusw_ap(k, m), rhs,
                                                 start=(k == 0), stop=(k == 3),
                                                 perf_mode=DR)
                        nc.vector.tensor_reduce(gmax[:, m, 4 * qb:4 * qb + 4],
                                                ps, axis=mybir.AxisListType.X,
                                                op=ALU.max)
                    nc.vector.tensor_scalar(gb[:, m, :], gmax[:, m, :],
                                            S_GB / (A_W * S_Z),
                                            const_col(CC_FUSC + m),
                                            op0=ALU.mult, op1=ALU.add)

                # pred1: 1280 -> 256 relu
                h1 = sn.tile([128, 2, PADQ, 128], fp8, tag="h1", name="h1")
                for m in range(2):
                    for qb in range(NQB):
                        ps = ppS.tile([128, 4, 128], f32, tag="psS", name="psS")
                        for qq in range(4):
                            q = 4 * qb + qq
                            for k in range(4):
                                off = zall.offset + (2 * k * PADQ + q) * PADV + 16
                                rhs = bass.AP(tensor=zall.tensor, offset=off,
                                              ap=[zall.ap[0],
                                                  [PADQ * PADV, 2], [1, 128]])
                                nc.tensor.matmul(ps[:, qq, :],
                                                 pw1_ap(k + 1, m), rhs,
                                                 start=(k == 0), stop=False,
                                                 perf_mode=DR)
                            rhs0 = bass.AP(tensor=gb.tensor,
                                           offset=gb.offset + q,
                                           ap=[gb.ap[0], [PADQ, 2], [0, 128]])
                            nc.tensor.matmul(ps[:, qq, :], pw1_ap(0, m),
                                             rhs0, start=False, stop=True,
                                             perf_mode=DR)
                        rr_relu(h1[:, m, 4 * qb:4 * qb + 4, :], ps,
                                S_H1 / (A_W * S_Z), const_col(CC_PB1 + m))

            # pred2 + pred3
            with tc.tile_pool(name="psumT", bufs=3, space="PSUM") as ppT:
                h2 = sn.tile([64, PADQ, 128], fp8, tag="h2", name="h2")
                for qb in range(NQB):
                    ps = ppT.tile([64, 4, 128], f32, tag="psT", name="psT")
                    for qq in range(4):
                        q = 4 * qb + qq
                        rhs = bass.AP(tensor=h1.tensor,
                                      offset=h1.offset + q * 128,
                                      ap=[h1.ap[0], [PADQ * 128, 2], [1, 128]])
                        nc.tensor.matmul(ps[:, qq, :], pw2_ap(), rhs,
                                         start=True, stop=True, perf_mode=DR)
                    rr_relu(h2[:, 4 * qb:4 * qb + 4, :], ps,
                            S_H2 / (A_W * S_H1), const_col(CC_PB2, parts=64))

                ps3 = ppT.tile([128, PADQ, 2], f32, tag="psT3", name="psT3",
                               bufs=1)
                for q in range(PADQ):
                    nc.tensor.matmul(ps3[:, q, :], h2[:, q, :], pw3_ap(),
                                     start=True, stop=True)
                o_t = sn.tile([128, PADQ, 2], f32, tag="o_t", name="o_t")
                HP = PADQ // 2
                nc.vector.scalar_tensor_tensor(
                    o_t[:, 0:HP, :], ps3[:, 0:HP, :], 1.0 / (A_W * S_H2),
                    bass.AP(tensor=base_ap.tensor, offset=base_ap.offset,
                            ap=[base_ap.ap[0], [2, HP], [1, 2]]),
                    op0=ALU.mult, op1=ALU.add)
                nc.sync.dma_start(out=d_out[:, 0:HP, :], in_=o_t[:, 0:HP, :])
                nc.vector.scalar_tensor_tensor(
                    o_t[:, HP:, :], ps3[:, HP:, :], 1.0 / (A_W * S_H2),
                    bass.AP(tensor=base_ap.tensor,
                            offset=base_ap.offset + HP * 2,
                            ap=[base_ap.ap[0], [2, PADQ - HP], [1, 2]]),
                    op0=ALU.mult, op1=ALU.add)
                nc.sync.dma_start(out=d_out[:, HP:, :], in_=o_t[:, HP:, :])

    nc.compile()
    return nc


_NC_CACHE = {}


def _get_nc_key(P, with_b2, zb):
    key = (P, with_b2, zb)
    if key not in _NC_CACHE:
        _NC_CACHE[key] = build_nc(P, with_b2, zb)
    return _NC_CACHE[key]


def _get_nc(P):
    """test.py compatibility: default flags for the standard input set."""
    return _get_nc_key(P, False, True)


def _host_prep(inputs, P, counts, order, offs):
    """Build per-core in_maps. Returns (in_maps, with_b2, zb)."""
    PADQ = -(-P // 4) * 4
    NV = PADQ * 128
    cnn = np.asarray(inputs["cnn_feature"], np.float32)
    ipoly = np.asarray(inputs["i_it_poly"], np.float32)
    cpoly = np.asarray(inputs["c_it_poly"], np.float32)
    w1 = np.asarray(inputs["proj_w1"], np.float32)
    pb0 = np.asarray(inputs["proj_b1"], np.float32)
    w2 = np.asarray(inputs["proj_w2"], np.float32)[:, :, 0, 0]  # [64, 256]
    b2 = np.asarray(inputs["proj_b2"], np.float32)
    head_w = np.asarray(inputs["head_w"], np.float32)   # [128, 66, 9]
    head_b = np.asarray(inputs["head_b"], np.float32)
    head_g = np.asarray(inputs["head_g"], np.float32)
    head_bt = np.asarray(inputs["head_bt"], np.float32)
    res_w = np.asarray(inputs["res_w"], np.float32)     # [7, 128, 128, 9]
    res_b = np.asarray(inputs["res_b"], np.float32)
    res_g = np.asarray(inputs["res_g"], np.float32)
    res_bt = np.asarray(inputs["res_bt"], np.float32)
    fus_w = np.asarray(inputs["fus_w"], np.float32)     # [256, 1024]
    fus_b = np.asarray(inputs["fus_b"], np.float32)
    pw1 = np.asarray(inputs["pw1"], np.float32)         # [256, 1280]
    pb1 = np.asarray(inputs["pb1"], np.float32)
    pw2 = np.asarray(inputs["pw2"], np.float32)         # [64, 256]
    pb2 = np.asarray(inputs["pb2"], np.float32)
    pw3 = np.asarray(inputs["pw3"], np.float32)         # [2, 64]
    pb3 = np.asarray(inputs["pb3"], np.float32)

    assert (head_g > 0).all() and (res_g > 0).all(), "bn fold requires g>0"
    with_b2 = bool(np.any(b2 != 0))

    # w1p [99, 3pairs, 2kt, 2m, 128]
    w1p = np.zeros((99, 3, 2, 2, 128), np.float32)
    pair_src = [((0, 0), (0, 1)), ((1, 0), (0, 2)), ((1, 1), (1, 2))]
    for p, pr in enumerate(pair_src):
        for kt, (blk, kw) in enumerate(pr):
            rm = ROWMAP_A if blk == 0 else ROWMAP_B
            for r in range(99):
                ch, kh = rm[r]
                for m in range(2):
                    w1p[r, p, kt, m, :] = A_W * w1[m * 128:(m + 1) * 128,
                                                   ch, kh, kw]
    pb0s = (S_R1 * pb0).reshape(2, 128).T.copy()

    w2t = np.zeros((128, 2, 64), np.float32)
    for kt in range(2):
        w2t[:, kt, :] = A_W * w2[:, kt * 128:(kt + 1) * 128].T

    # ---- grid-sample host math ----
    ix = ipoly[..., 0] - np.float32(0.5)
    iy = ipoly[..., 1] - np.float32(0.5)
    x0 = np.floor(ix); y0 = np.floor(iy)
    wx = (ix - x0).astype(np.float32); wy = (iy - y0).astype(np.float32)
    x0i = x0.astype(np.int64); y0i = y0.astype(np.int64)

    swap_x = x0i < 0
    vx0 = (x0i >= 0) & (x0i < W)
    vx1 = (x0i + 1 >= 0) & (x0i + 1 < W)

    def slot_weights(yi):
        vy = (yi >= 0) & (yi < H)
        w_s0 = (1 - wx) * vx0 * vy
        w_s1 = wx * vx1 * vy
        w_s0 = np.where(swap_x, wx * vx1 * vy, w_s0)
        w_s1 = np.where(swap_x, 0.0, w_s1)
        return w_s0.astype(np.float32), w_s1.astype(np.float32)

    x0c = np.clip(x0i, 0, W - 2)
    y0c = np.clip(y0i, 0, H - 1)
    y1c = np.clip(y0i + 1, 0, H - 1)
    idxA = (y0c * W + x0c).astype(np.int64)          # [NP, V]
    idxB = (y1c * W + x0c).astype(np.int64)
    wA0, wA1 = slot_weights(y0i)
    wB0, wB1 = slot_weights(y0i + 1)
    wA0 *= (1 - wy); wA1 *= (1 - wy)
    wB0 *= wy; wB1 *= wy
    s_v = wA0 + wA1 + wB0 + wB1

    # ---- snake weights (bn + scale folds) ----
    headw = np.zeros((66, 5, 2, 128), np.float32)
    hw9 = head_w.transpose(1, 2, 0)                  # [66, 9, 128]
    for t in range(9):
        p, kt = t // 2, t % 2
        headw[0:64, p, kt, :] = A_W * hw9[0:64, t, :]
        headw[64:66, p, kt, :] = A_W * S_FEAT * hw9[64:66, t, :]
    lsb = np.zeros((128, 8, 2), np.float32)
    lsb[:, 0, 0] = head_g / A_W
    lsb[:, 0, 1] = S_Z * head_g * head_b
    C = np.zeros((8, 128), np.float32)               # C_i = sum_{j<=i} bt_j
    C[0] = head_bt
    for i in range(NRES):
        C[i + 1] = C[i] + res_bt[i]

    resw = np.zeros((128, 7, 5, 2, 128), np.float32)
    for i in range(NRES):
        rw = res_w[i].transpose(1, 2, 0)             # [128 in, 9, 128 out]
        for t in range(9):
            p, kt = t // 2, t % 2
            resw[:, i, p, kt, :] = A_W * rw[:, t, :]
        bprime = res_b[i] + res_w[i].sum(axis=2) @ C[i]
        lsb[:, i + 1, 0] = res_g[i] / A_W
        lsb[:, i + 1, 1] = S_Z * res_g[i] * bprime

    fw8 = fus_w.reshape(256, 8, 128)
    fusw = np.zeros((128, 4, 2, 2, 128), np.float32)
    for k in range(4):
        for kt in range(2):
            for m in range(2):
                fusw[:, k, kt, m, :] = A_W * fw8[m * 128:(m + 1) * 128,
                                                 2 * k + kt].T
    fusconst = fus_b + np.einsum('ojc,jc->o', fw8, C)
    fusc = (S_GB * fusconst).reshape(2, 128).T.copy()

    pw1r = pw1.reshape(256, 10, 128)
    pw1p = np.zeros((128, 5, 2, 2, 128), np.float32)
    for m in range(2):
        for kt in range(2):
            pw1p[:, 0, kt, m, :] = (A_W * S_Z / S_GB) * \
                pw1r[m * 128:(m + 1) * 128, kt].T
        for k in range(4):
            for kt in range(2):
                pw1p[:, k + 1, kt, m, :] = A_W * \
                    pw1r[m * 128:(m + 1) * 128, 2 + 2 * k + kt].T
    pb1prime = pb1 + np.einsum('ojc,jc->o', pw1r[:, 2:], C)
    pb1s = (S_H1 * pb1prime).reshape(2, 128).T.copy()

    pw2t = np.zeros((128, 2, 64), np.float32)
    for kt in range(2):
        pw2t[:, kt, :] = A_W * pw2[:, kt * 128:(kt + 1) * 128].T
    pb2s = (S_H2 * pb2).reshape(64, 1)
    pw3t = A_W * pw3.T                                # [64, 2]

    zb = (not np.any(pb0)) and (not np.any(lsb[:, :, 1])) \
        and (not np.any(pb1s)) and (not np.any(pb2s))

    # ---- pack blobs ----
    CA = 2946
    blobA = np.zeros((128, CA), F8)
    blobA[0:99, 0:1536] = _f8(w1p).reshape(99, -1)
    blobA[:, 1536:1664] = _f8(w2t).reshape(128, -1)
    blobA[0:66, 1664:2944] = _f8(headw).reshape(66, -1)
    blobA[0:64, 2944:2946] = _f8(pw3t)

    CB_RES = PADQ * 512
    CB_FUS = CB_RES + 8960
    CB_PW1 = CB_FUS + 2048
    CB_PW2 = CB_PW1 + 2560
    CB = CB_PW2 + 128
    blobB_shared = np.zeros((128, CB - CB_RES), F8)
    blobB_shared[:, 0:8960] = _f8(resw).reshape(128, -1)
    blobB_shared[:, 8960:8960 + 2048] = _f8(fusw).reshape(128, -1)
    blobB_shared[:, 11008:11008 + 2560] = _f8(pw1p).reshape(128, -1)
    pw2pad = np.zeros((128, 2, 64), np.float32)
    pw2pad[:, :, :] = pw2t[:, :, 0:64]
    blobB_shared[:, 13568:13696] = _f8(pw2pad).reshape(128, -1)

    CC = 24 + PADQ * 2
    consts_shared = np.zeros((128, CC), np.float32)
    consts_shared[:, 0:2] = pb0s
    consts_shared[:, 2:18] = lsb.reshape(128, -1)
    consts_shared[:, 18:20] = fusc
    consts_shared[:, 20:22] = pb1s
    consts_shared[0:64, 22:23] = pb2s

    shared = {"blobA": blobA}
    if with_b2:
        shared["b2r"] = _f8(S_FEAT * b2.reshape(1, 64))

    in_maps = []
    for c in range(N_CORES):
        img = cnn[c]
        img_pad = np.zeros((C_IN, PADW, PADW), np.float32)
        img_pad[:, 1:129, 1:129] = img
        flatf = _f8(img_pad.reshape(C_IN, PIMG)).astype(np.float32)
        stk = np.zeros((99, PIMG, 2), np.float32)
        for r in range(99):
            ch, kh = ROWMAP_A[r]
            ln = PIMG - kh * PADW
            stk[r, :ln, 0] = flatf[ch, kh * PADW:]
            ch, kh = ROWMAP_B[r]
            ln = PIMG - kh * PADW
            stk[r, :ln, 1] = flatf[ch, kh * PADW:]
        stk = stk.reshape(99, 2 * PIMG)

        own = order[offs[c]:offs[c + 1]]
        nown = len(own)
        gixa = np.zeros(NV, np.int64)
        gixb = np.zeros(NV, np.int64)
        dgt = np.zeros((128, PADQ, 4, 128), np.float32)
        ar = np.arange(128)
        for qi, poly in enumerate(own):
            gixa[qi * 128:(qi + 1) * 128] = idxA[poly]
            gixb[qi * 128:(qi + 1) * 128] = idxB[poly]
            dgt[ar, qi, 0, ar] = wA0[poly]
            dgt[ar, qi, 1, ar] = wA1[poly]
            dgt[ar, qi, 2, ar] = wB0[poly]
            dgt[ar, qi, 3, ar] = wB1[poly]

        cpv = np.zeros((2, PADQ, 160), np.float32)
        if nown:
            cc = (cpoly[own] * RO).transpose(2, 0, 1)     # [2, nown, 128]
            cpv[:, :nown, 16:144] = cc
            cpv[:, :nown, 0:16] = cc[:, :, 112:128]
            cpv[:, :nown, 144:160] = cc[:, :, 0:16]

        base = np.zeros((128, PADQ, 2), np.float32)
        if nown:
            base[:, :nown, :] = (ipoly[own] * RO + pb3[None, None, :]) \
                .transpose(1, 0, 2)

        blobB = np.zeros((128, CB), F8)
        blobB[:, 0:CB_RES] = _f8(dgt).reshape(128, -1)
        blobB[:, CB_RES:] = blobB_shared
        consts = consts_shared.copy()
        consts[:, 24:] = base.reshape(128, -1)
        gix = np.concatenate([pack16(gixa, NV // 16),
                              pack16(gixb, NV // 16)], axis=1)
        m = {
            "stk": _f8(stk), "blobB": blobB, "consts": consts,
            "gix": gix, "cpv": _f8(cpv),
        }
        if with_b2:
            svr = np.zeros((1, PADQ, 128), np.float32)
            svr[0, :nown, :] = s_v[own]
            m["svr"] = _f8(svr)
        m.update(shared)
        in_maps.append(m)
    return in_maps, with_b2, zb


def kernel(**inputs):
    ind = np.asarray(inputs["ind"]).astype(np.int64)
    counts = np.bincount(ind, minlength=N_CORES)
    P = int(counts.max())
    order = np.argsort(ind, kind="stable")
    offs = np.concatenate([[0], np.cumsum(counts)])

    in_maps, with_b2, zb = _host_prep(inputs, P, counts, order, offs)
    nc = _get_nc_key(P, with_b2, zb)
    res = None
    last_err = None
    for _attempt in range(3):
        try:
            res = run_bass_kernel_spmd(nc, in_maps, list(range(N_CORES)))
            break
        except Exception as e:  # rare transient device error; retry
            last_err = e
    if res is None:
        raise last_err

    out = np.zeros((NP, V, 2), np.float32)
    for c in range(N_CORES):
        oc = res.results[c]["out"]  # [128v, PADQ, 2]
        own = order[offs[c]:offs[c + 1]]
        for q, opoly in enumerate(own):
            out[opoly] = oc[:, q, :]
    return out
